# revision 1
# baseline (speedup 1.0000x reference)
"""Multi-head self-attention Trainium2 kernel (8 NeuronCores).

Sharding: 8 cores = 4 batches x 2 head-halves (Megatron-style tensor
parallel over heads within a batch). Each core computes, for its batch b
and its 6 heads:
    Q^T, K^T (head-dim-on-partitions layout), V (natural layout, one
    ones-column per head for the softmax denominator), S^T = K.Q^T per
    (head, q-block) with the two heads' K=64 matmuls row-packed into
    opposite halves of the PE array, P = exp(S/8) on ScalarE straight
    out of 3-bank PSUM groups (softmax without max subtraction --
    scores are O(5), safe in fp32), ctx^T = (V^T P^T) accumulated in
    PSUM with the denominator arriving as the ones-row, normalization
    fused into the PSUM drain, and the out-projection
    Y_partial = ctx @ Wo[:, cols]^T.
The host sums the two per-batch partials and adds the output bias.

Scheduling: each unit's ctx matmuls trail one unit behind the
score/exp stream; the next head pair's Q/K projections and the
out-projection slices are dripped into the rotation stream so the
ScalarE exp pipeline (the saturating engine) never starves.

All matmul operands are bf16 (fp32 PSUM accumulation); exp and
normalization run in fp32.
"""

import numpy as np

B, S, E, H, D = 4, 2048, 768, 12, 64
NCORES = 8

_FULL_CFG = dict(S=2048, E=768, EL=384)

_cache = {}


def _emit(nc, tc, ctx, io, cfg):
    import concourse.mybir as mybir

    fp32 = mybir.dt.float32
    bf16 = mybir.dt.bfloat16
    Exp = mybir.ActivationFunctionType.Exp

    Scfg, Ecfg, EL = cfg["S"], cfg["E"], cfg["EL"]
    ROT = cfg.get("rot", 3)    # exp group size in PSUM banks
    NKT = Ecfg // 128          # contraction tiles over embed dim
    NS = Scfg // 128           # sequence tiles (also key tiles)
    NQB = Scfg // 512          # query blocks
    HP = EL // 128             # head pairs (2 heads each)
    NCOMBO = 2 * NS            # (head, k-tile) combos per unit
    NROT = (NCOMBO + ROT - 1) // ROT   # exp rotations
    NYC = (Ecfg + 383) // 384  # out-proj column chunks
    YCW = Ecfg // NYC          # chunk width
    scale = 1.0 / np.sqrt(D)

    XT, WQT, WKT, WVT, WOT, BQ, BK, BV, Y = (
        io["XT"], io["WQT"], io["WKT"], io["WVT"], io["WOT"],
        io["BQ"], io["BK"], io["BV"], io["Y"],
    )

    consts = ctx.enter_context(tc.tile_pool(name="consts", bufs=1))
    wpool = ctx.enter_context(tc.tile_pool(name="wpool", bufs=1))
    xpool = ctx.enter_context(tc.tile_pool(name="xpool", bufs=1))
    qkpool = ctx.enter_context(tc.tile_pool(name="qkpool", bufs=3))
    vpool = ctx.enter_context(tc.tile_pool(name="vpool", bufs=1))
    spool = ctx.enter_context(tc.tile_pool(name="spool", bufs=22))
    cpool = ctx.enter_context(tc.tile_pool(name="cpool", bufs=1))
    rpool = ctx.enter_context(tc.tile_pool(name="rpool", bufs=4))
    ypool = ctx.enter_context(tc.tile_pool(name="ypool", bufs=4))
    psum_g = ctx.enter_context(tc.tile_pool(name="psum_g", bufs=2, space="PSUM"))
    psum_c = ctx.enter_context(tc.tile_pool(name="psum_c", bufs=1, space="PSUM"))
    psum_m = ctx.enter_context(tc.tile_pool(name="psum_m", bufs=1, space="PSUM"))

    # ---- constants & weights to SBUF (critical-path tensors first: the
    # first projection chain needs all XT tiles plus Wq/Wk) ----
    xt_sb = []
    for kk in range(NKT):
        t = xpool.tile([128, Scfg], bf16, name=f"xt{kk}_sb", tag=f"xt{kk}")
        nc.sync.dma_start(out=t, in_=XT[kk * 128:(kk + 1) * 128, :])
        xt_sb.append(t)

    wq_sb, wk_sb, wv_sb = [], [], []
    for kk in range(NKT):
        for lst, src, nm in ((wq_sb, WQT, "wq"), (wk_sb, WKT, "wk"), (wv_sb, WVT, "wv")):
            t = wpool.tile([128, EL], bf16, name=f"{nm}{kk}_sb", tag=f"{nm}{kk}")
            nc.sync.dma_start(out=t, in_=src[kk * 128:(kk + 1) * 128, :])
            lst.append(t)

    bq_sb = consts.tile([128, HP], fp32, name="bq_sb")
    nc.sync.dma_start(out=bq_sb, in_=BQ)
    bk_sb = consts.tile([128, HP], fp32, name="bk_sb")
    nc.sync.dma_start(out=bk_sb, in_=BK)
    bv_sb = consts.tile([128, EL], fp32, name="bv_sb")
    nc.sync.dma_start(out=bv_sb, in_=BV)

    wo_sb = []
    for hp in range(HP):
        t = wpool.tile([128, Ecfg], bf16, name=f"wo{hp}_sb", tag=f"wo{hp}")
        nc.sync.dma_start(out=t, in_=WOT[hp * 128:(hp + 1) * 128, :])
        wo_sb.append(t)

    ctxT = []
    for hp in range(HP):
        t = cpool.tile([128, Scfg], bf16, name=f"ctxT{hp}", tag=f"ctx{hp}")
        ctxT.append(t)

    # ctx for unit u runs while unit u+1's score rotations stream, keeping
    # ACT saturated. Combos are k-major (ci = 2k+hh) so the two heads' K=64
    # score matmuls land in opposite row-halves of the PE array and run
    # concurrently.
    def ctx_combo(u, idx):
        hh, k = divmod(idx, NS)
        ci = 2 * k + hh
        if k == 0:
            u["cps"] = psum_c.tile([128, 512], fp32,
                                   name=f"cps{u['hp']}_{u['qb']}_{hh}", tag="c")
        cps = u["cps"]
        vt = v_sb[k]
        off = (2 * u["hp"] + hh) * 65
        rhs = u["slabs"][ci // ROT][:, (ci % ROT) * 512:(ci % ROT) * 512 + 512]
        nc.tensor.matmul(cps[0:65, :], lhsT=vt[:, off:off + 65],
                         rhs=rhs, start=(k == 0), stop=(k == NS - 1))
        if k == NS - 1:
            hp_u, qb_u = u["hp"], u["qb"]
            qsl = slice(qb_u * 512, qb_u * 512 + 512)
            r_sb = rpool.tile([1, 512], fp32, name=f"r{hp_u}_{qb_u}_{hh}", tag="r")
            nc.vector.reciprocal(r_sb, cps[64:65, :])
            rb = rpool.tile([128, 512], fp32, name=f"rb{hp_u}_{qb_u}_{hh}", tag="rb")
            nc.gpsimd.partition_broadcast(rb, r_sb)
            nc.vector.tensor_mul(
                ctxT[hp_u][64 * hh:64 * hh + 64, qsl],
                cps[0:64, :], rb[0:64, :])

    prev_unit = None
    import collections
    pending = collections.deque()

    def ptag(pool):
        if pool is psum_m:
            return "m"
        return "c" if pool is psum_c else "g"

    dripcnt = [0]

    def drip(n=1, alt=False, pools=None):
        for _ in range(min(n, len(pending))):
            if pools is not None:
                pool = pools[dripcnt[0] % len(pools)]
            else:
                pool = psum_c if (alt and dripcnt[0] % 2 == 1) else psum_m
            dripcnt[0] += 1
            pending.popleft()(pool)

    # ---- projection closures (one PSUM slice each) so they can be dripped
    # into the attention rotation stream of the previous head pair ----
    def build_proj(hp):
        qt = qkpool.tile([128, Scfg], bf16, name=f"qt{hp}", tag="qt")
        kt = qkpool.tile([128, Scfg], bf16, name=f"kt{hp}", tag="kt")
        closures = []
        for lbl, dst, wsb, bsb in (("q", qt, wq_sb, bq_sb), ("k", kt, wk_sb, bk_sb)):
            for nb in range(NQB):
                def cl(pool, lbl=lbl, dst=dst, wsb=wsb, bsb=bsb, nb=nb, hp=hp):
                    ps = pool.tile([128, 512], fp32,
                                   name=f"pj{hp}{lbl}_{nb}", tag=ptag(pool))
                    for kk in range(NKT):
                        nc.tensor.matmul(
                            ps, lhsT=wsb[kk][:, hp * 128:(hp + 1) * 128],
                            rhs=xt_sb[kk][:, nb * 512:(nb + 1) * 512],
                            start=(kk == 0), stop=(kk == NKT - 1))
                    nc.vector.tensor_scalar_add(
                        dst[:, nb * 512:(nb + 1) * 512], ps, bsb[:, hp:hp + 1])
                closures.append(cl)
        return qt, kt, closures

    # ---- V for ALL heads at once: one (128, 65*2*HP) tile per s-tile,
    # [d0..d63 | 1] per head, consumed by every head pair's ctx ----
    v_sb = [vpool.tile([128, 65 * 2 * HP], bf16, name=f"v_{si}", tag=f"v{si}")
            for si in range(NS)]

    def build_v():
        closures = []
        for si in range(NS):
            def cl(pool, si=si, vt=v_sb[si]):
                nc.vector.memset(vt, 1.0)
                ps = pool.tile([128, EL], fp32, name=f"pv{si}",
                               tag=ptag(pool))
                for kk in range(NKT):
                    nc.tensor.matmul(
                        ps, lhsT=xt_sb[kk][:, si * 128:(si + 1) * 128],
                        rhs=wv_sb[kk],
                        start=(kk == 0), stop=(kk == NKT - 1))
                nc.vector.tensor_add(
                    vt.rearrange("p (h w) -> p h w", w=65)[:, :, 0:64],
                    ps.rearrange("p (h w) -> p h w", w=64),
                    bv_sb.rearrange("p (h w) -> p h w", w=64))
            closures.append(cl)
        return closures

    # ---- out-projection closures: Y = ctx @ Wo_loc^T, dripped into the
    # last head pair's attention stream once the needed ctxT columns are
    # fully drained ----
    def build_y(si):
        def cl(pool, pool2=None, si=si):
            y_sb = ypool.tile([128, Ecfg], fp32, name=f"y{si}", tag="y")
            for nh in range(NYC):
                p = pool if (pool2 is None or nh % 2 == 0) else pool2
                yps = p.tile([128, YCW], fp32, name=f"yp{si}_{nh}",
                             tag=ptag(p))
                for hp in range(HP):
                    nc.tensor.matmul(
                        yps, lhsT=ctxT[hp][:, si * 128:(si + 1) * 128],
                        rhs=wo_sb[hp][:, nh * YCW:(nh + 1) * YCW],
                        start=(hp == 0), stop=(hp == HP - 1))
                nc.vector.tensor_copy(y_sb[:, nh * YCW:(nh + 1) * YCW], yps)
            nc.sync.dma_start(out=Y[si * 128:(si + 1) * 128, :], in_=y_sb)
        return cl

    y_by_qb = {qb: [build_y(si) for si in range(qb * (NS // NQB),
                                                (qb + 1) * (NS // NQB))]
               for qb in range(NQB)}

    y_sched = set()
    cur = build_proj(0)
    vcl = build_v()
    for hp in range(HP):
        qt, kt, closures = cur
        if hp == 0:
            # minimal prefix so unit (0,0) can start: its own q slice plus
            # the first key block (rotation r only reads keys < ~192*(r+1),
            # so later kt slices can trail behind via subtile deps). The
            # rest drips into the first unit's rotation stream on both
            # spare PSUM banks, keys first.
            head = [closures[0], closures[NQB]]
            for i, cl in enumerate(head):
                cl(psum_m if i % 2 == 0 else psum_c)
            krest = closures[NQB + 1:2 * NQB]
            qrest = closures[1:NQB]
            nv = len(vcl)
            if qrest:
                pend0 = (krest + [qrest[0]] + vcl[:nv // 2] + qrest[1:2]
                         + vcl[nv // 2:] + qrest[2:])
            else:
                pend0 = krest + list(vcl)
            pending.extend(pend0)
        cur = build_proj(hp + 1) if hp + 1 < HP else None
        if cur is not None:
            pending.extend(cur[2])

        for qb in range(NQB):
            qsl = slice(qb * 512, qb * 512 + 512)
            if hp == HP - 1 and qb >= 2:
                y_sched.add(qb - 2)
                pending.extend(y_by_qb[qb - 2])
            unit = dict(hp=hp, qb=qb, slabs=[], cps=None, cur=0)
            first_unit = (hp == 0 and qb == 0)
            for r in range(NROT):
                cis = range(ROT * r, min(ROT * r + ROT, NCOMBO))
                n = len(cis)
                g = psum_g.tile([128, 512 * ROT], fp32,
                                name=f"g{hp}_{qb}_{r}", tag="g")
                for j, ci in enumerate(cis):
                    hh, k = ci % 2, ci // 2
                    nc.tensor.matmul(
                        g[:, j * 512:(j + 1) * 512],
                        lhsT=kt[hh * 64:(hh + 1) * 64, k * 128:(k + 1) * 128],
                        rhs=qt[hh * 64:(hh + 1) * 64, qsl],
                        start=True, stop=True)
                slab = spool.tile([128, 512 * ROT], bf16,
                                  name=f"s{hp}_{qb}_{r}", tag="slab")
                nc.scalar.activation(slab[:, :n * 512], g[:, :n * 512], Exp,
                                     scale=float(scale))
                unit["slabs"].append(slab)
                if prev_unit is not None:
                    target = min(NCOMBO, ROT * (r + 1))
                    while prev_unit["cur"] < target:
                        ctx_combo(prev_unit, prev_unit["cur"])
                        prev_unit["cur"] += 1
                drip(2 if first_unit else 1, alt=first_unit)
            if prev_unit is not None:
                while prev_unit["cur"] < NCOMBO:
                    ctx_combo(prev_unit, prev_unit["cur"])
                    prev_unit["cur"] += 1
            prev_unit = unit

    # drain the last unit's ctx, dripping in remaining out-projections
    if NQB >= 2:
        y_sched.add(NQB - 2)
        pending.extend(y_by_qb[NQB - 2])
    if prev_unit is not None:
        while prev_unit["cur"] < NCOMBO:
            ctx_combo(prev_unit, prev_unit["cur"])
            prev_unit["cur"] += 1
            if prev_unit["cur"] % 2 == 0:
                drip(1, pools=[psum_m, psum_g])
    tail = [cl for qb in range(NQB) if qb not in y_sched
            for cl in y_by_qb[qb]]
    cyc = [psum_m, psum_g, psum_c, psum_g]
    for i, cl in enumerate(tail):
        cl(cyc[i % 4], cyc[(i + 1) % 4])
    drip(len(pending))


def _build(cfg):
    import contextlib
    import concourse.mybir as mybir
    import concourse.tile as tile
    from concourse import bacc

    Scfg, Ecfg, EL = cfg["S"], cfg["E"], cfg["EL"]
    HP = EL // 128
    fp32, bf16 = mybir.dt.float32, mybir.dt.bfloat16

    nc = bacc.Bacc("TRN2", target_bir_lowering=False, debug=False,
                   num_devices=NCORES)
    io = {
        "XT": nc.dram_tensor("XT", [Ecfg, Scfg], bf16, kind="ExternalInput").ap(),
        "WQT": nc.dram_tensor("WQT", [Ecfg, EL], bf16, kind="ExternalInput").ap(),
        "WKT": nc.dram_tensor("WKT", [Ecfg, EL], bf16, kind="ExternalInput").ap(),
        "WVT": nc.dram_tensor("WVT", [Ecfg, EL], bf16, kind="ExternalInput").ap(),
        "WOT": nc.dram_tensor("WOT", [EL, Ecfg], bf16, kind="ExternalInput").ap(),
        "BQ": nc.dram_tensor("BQ", [128, HP], fp32, kind="ExternalInput").ap(),
        "BK": nc.dram_tensor("BK", [128, HP], fp32, kind="ExternalInput").ap(),
        "BV": nc.dram_tensor("BV", [128, EL], fp32, kind="ExternalInput").ap(),
        "Y": nc.dram_tensor("Y", [Scfg, Ecfg], fp32, kind="ExternalOutput").ap(),
    }
    with tile.TileContext(nc) as tc:
        with contextlib.ExitStack() as ctx:
            _emit(nc, tc, ctx, io, cfg)
    nc.compile()
    return nc


def _get_program(cfg_key="full"):
    if cfg_key not in _cache:
        _cache[cfg_key] = _build(_FULL_CFG)
    return _cache[cfg_key]


def _half_inputs(half, Wq, bq, Wk, bk, Wv, bv, Wo=None):
    """Per-head-half tensors, shared by the 4 cores of that half."""
    import ml_dtypes
    bf16 = ml_dtypes.bfloat16
    e0 = 384 * half
    ecols = slice(e0, e0 + 384)
    return {
        **({"WOT": np.ascontiguousarray(Wo[:, ecols].T).astype(bf16)}
           if Wo is not None else {}),
        "WQT": np.ascontiguousarray(Wq[ecols, :].T).astype(bf16),
        "WKT": np.ascontiguousarray(Wk[ecols, :].T).astype(bf16),
        "WVT": np.ascontiguousarray(Wv[ecols, :].T).astype(bf16),
        "BQ": np.ascontiguousarray(bq[ecols].reshape(3, 128).T).astype(np.float32),
        "BK": np.ascontiguousarray(bk[ecols].reshape(3, 128).T).astype(np.float32),
        "BV": np.ascontiguousarray(
            np.broadcast_to(bv[ecols], (128, 384))).astype(np.float32),
    }


def _core_inputs(c, X, Wq, bq, Wk, bk, Wv, bv):
    """Kept for the test harness: full per-core input dict (includes the
    half-shared tensors, minus WOT)."""
    import ml_dtypes
    b, half = divmod(c, 2)
    m = _half_inputs(half, Wq, bq, Wk, bk, Wv, bv)
    m["XT"] = np.ascontiguousarray(X[b].T).astype(ml_dtypes.bfloat16)
    return m


def kernel(X, Wq, bq, Wk, bk, Wv, bv, Wo, bo):
    import ml_dtypes
    from concourse.bass_utils import run_bass_kernel_spmd

    bf16 = ml_dtypes.bfloat16
    X, Wq, bq, Wk, bk, Wv, bv, Wo, bo = [
        np.asarray(a, dtype=np.float32)
        for a in (X, Wq, bq, Wk, bk, Wv, bv, Wo, bo)
    ]
    nc = _get_program()
    halves = [_half_inputs(h, Wq, bq, Wk, bk, Wv, bv, Wo) for h in range(2)]
    xts = [np.ascontiguousarray(X[b].T).astype(bf16) for b in range(B)]
    in_maps = [dict(halves[c % 2], XT=xts[c // 2]) for c in range(NCORES)]
    res = run_bass_kernel_spmd(nc, in_maps, list(range(NCORES)))
    out = np.empty((B, S, E), np.float32)
    for b in range(B):
        out[b] = (res.results[2 * b]["Y"] + res.results[2 * b + 1]["Y"]
                  + bo[None, :])
    return out



# revision 34
# speedup vs baseline: 1.1463x; 1.1463x over previous
"""Multi-head self-attention Trainium2 kernel (8 NeuronCores).

Sharding: 8 cores = 4 batches x 2 head-halves (6 heads each). Per core:
QKV projections run as fp8(e4m3) hi+lo DoubleRow matmuls (host splits X
and W into hi/lo fp8 pairs; the lo*lo term is dropped), scores S^T = K.Q^T
in fp16 per (head, key-tile, q-block), P = exp(S/8) on ScalarE straight
out of 3-bank PSUM groups, ctx in the natural [q, d] orientation
(lhsT = P^T slab slices, rhs = V with a ones column for the softmax
denominator) accumulated per (head, q-block) in a time-split PSUM bank,
normalization + PE transpose into ctx^T, and the out-projection
Y_partial = ctx @ Wo[:, cols]^T in fp16. The host sums the two per-batch
partials and adds the output bias.

Scheduling: each unit's ctx matmuls trail one unit behind the score/exp
stream (head-major combo order so the shared ctx PSUM bank is time-split
between the two heads); projections for the next head pair, V tiles,
transposes and out-projection slices drip into the rotation stream.
"""

import numpy as np

B, S, E, H, D = 4, 2048, 768, 12, 64
NCORES = 8

_cache = {}


def _emit(nc, tc, ctx, io):
    import concourse.mybir as mybir

    fp32 = mybir.dt.float32
    fp16 = mybir.dt.float16
    fp8 = mybir.dt.float8e4
    i16 = mybir.dt.int16
    Exp = mybir.ActivationFunctionType.Exp
    DR = mybir.MatmulPerfMode.DoubleRow

    Scfg, Ecfg, EL = 2048, 768, 384
    NKT = Ecfg // 128           # contraction tiles over embed dim (6)
    NS = Scfg // 128            # key tiles (16)
    NQB = Scfg // 512           # q-blocks (4)
    HP = EL // 128              # head pairs (3)
    ROT = 3                     # exp group size in PSUM banks
    NCOMBO = 2 * NS             # (head, k-tile) combos per unit (32)
    NROT = (NCOMBO + ROT - 1) // ROT  # 11
    SCH = ()                    # rotation indices offloaded to DVE exp-approx
    SCH_A = float(1024.0 * np.log2(np.e) / 8.0)
    SCH_B = float(1024.0 * (15.0 + 0.045) + 0.5)

    X8, W8Q, W8QS, W8K, W8KS, W8V, W8VS, WOT, BQ, BK, BV, IDT, Y = (
        io["X8"], io["W8Q"], io["W8QS"], io["W8K"], io["W8KS"], io["W8V"],
        io["W8VS"], io["WOT"], io["BQ"], io["BK"], io["BV"], io["IDT"],
        io["Y"],
    )

    consts = ctx.enter_context(tc.tile_pool(name="consts", bufs=1))
    wpool = ctx.enter_context(tc.tile_pool(name="wpool", bufs=1))
    xpool = ctx.enter_context(tc.tile_pool(name="xpool", bufs=1))
    qkpool = ctx.enter_context(tc.tile_pool(name="qkpool", bufs=4))
    vpool = ctx.enter_context(tc.tile_pool(name="vpool", bufs=1))
    spool = ctx.enter_context(tc.tile_pool(name="spool", bufs=22))
    cpool = ctx.enter_context(tc.tile_pool(name="cpool", bufs=1))
    npool = ctx.enter_context(tc.tile_pool(name="npool", bufs=4))
    ypool = ctx.enter_context(tc.tile_pool(name="ypool", bufs=2))
    psum_g = ctx.enter_context(tc.tile_pool(name="psum_g", bufs=2, space="PSUM"))
    psum_c = ctx.enter_context(tc.tile_pool(name="psum_c", bufs=1, space="PSUM"))
    psum_m = ctx.enter_context(tc.tile_pool(name="psum_m", bufs=1, space="PSUM"))

    # ---- weights / constants to SBUF (critical-path tensors first) ----
    # DMA issue order = first-projection critical path: weights+chunk0,
    # then the (tiny) bias the first drain needs, then the cross weights
    w8q_sb = wpool.tile([128, NKT * 2 * EL], fp8, name="w8q_sb")
    nc.sync.dma_start(out=w8q_sb, in_=W8Q)
    w8k_sb = wpool.tile([128, NKT * 2 * EL], fp8, name="w8k_sb")
    nc.sync.dma_start(out=w8k_sb, in_=W8K)
    x8_sb = xpool.tile([128, NKT * 2 * Scfg], fp8, name="x8_sb")
    x4s = x8_sb.rearrange("p (k h t) -> p k h t", k=NKT, h=2)
    x4d = X8.rearrange("p (k h t) -> p k h t", k=NKT, h=2)

    def xchunk(tc_):
        tsl = slice(tc_ * 512, (tc_ + 1) * 512)
        nc.sync.dma_start(out=x4s[:, :, :, tsl], in_=x4d[:, :, :, tsl])

    xchunk(0)
    bq_sb = consts.tile([128, HP], fp32, name="bq_sb")
    nc.sync.dma_start(out=bq_sb, in_=BQ)
    bk_sb = consts.tile([128, HP], fp32, name="bk_sb")
    nc.sync.dma_start(out=bk_sb, in_=BK)
    w8qs_sb = wpool.tile([128, NKT * 2 * EL], fp8, name="w8qs_sb")
    nc.sync.dma_start(out=w8qs_sb, in_=W8QS)
    w8ks_sb = wpool.tile([128, NKT * 2 * EL], fp8, name="w8ks_sb")
    nc.sync.dma_start(out=w8ks_sb, in_=W8KS)
    for tc_ in range(1, 4):
        xchunk(tc_)
    bv_sb = consts.tile([128, EL], fp32, name="bv_sb")
    nc.sync.dma_start(out=bv_sb, in_=BV)
    idt_sb = consts.tile([128, 128], fp16, name="idt_sb")
    nc.sync.dma_start(out=idt_sb, in_=IDT)
    w8v_sb = wpool.tile([128, NKT * 2 * EL], fp8, name="w8v_sb")
    nc.sync.dma_start(out=w8v_sb, in_=W8V)
    w8vs_sb = wpool.tile([128, NKT * 2 * EL], fp8, name="w8vs_sb")
    nc.sync.dma_start(out=w8vs_sb, in_=W8VS)

    wo_sb = []
    for hp in range(HP):
        t = wpool.tile([128, Ecfg], fp16, name=f"wo{hp}_sb", tag=f"wo{hp}")
        nc.sync.dma_start(out=t, in_=WOT[hp * 128:(hp + 1) * 128, :])
        wo_sb.append(t)

    # 4D views: [p, ktile, hi/lo, cols]
    x4 = x8_sb.rearrange("p (k h t) -> p k h t", k=NKT, h=2)
    w4q = w8q_sb.rearrange("p (k h c) -> p k h c", k=NKT, h=2)
    w4qs = w8qs_sb.rearrange("p (k h c) -> p k h c", k=NKT, h=2)
    w4k = w8k_sb.rearrange("p (k h c) -> p k h c", k=NKT, h=2)
    w4ks = w8ks_sb.rearrange("p (k h c) -> p k h c", k=NKT, h=2)
    w4v = w8v_sb.rearrange("p (k h c) -> p k h c", k=NKT, h=2)
    w4vs = w8vs_sb.rearrange("p (k h c) -> p k h c", k=NKT, h=2)

    ctxT = []
    for hp in range(HP):
        t = cpool.tile([128, Scfg], fp16, name=f"ctxT{hp}", tag=f"ctx{hp}")
        ctxT.append(t)

    import collections
    pending = collections.deque()

    def drip(n=1):
        for _ in range(min(n, len(pending))):
            pending.popleft()()

    # ---- fp8 hi/lo DoubleRow projection closures ----
    def build_proj(hp):
        qt = qkpool.tile([128, Scfg], fp16, name=f"qt{hp}", tag="qt")
        kt = qkpool.tile([128, Scfg], fp16, name=f"kt{hp}", tag="kt")
        hsl = slice(hp * 128, (hp + 1) * 128)
        closures = []
        for dst, w4, w4s, bsb in ((qt, w4q, w4qs, bq_sb), (kt, w4k, w4ks, bk_sb)):
            for nb in range(NQB):
                def cl(dst=dst, w4=w4, w4s=w4s, bsb=bsb, nb=nb, hp=hp, hsl=hsl):
                    tb = slice(nb * 512, (nb + 1) * 512)
                    ps = psum_m.tile([128, 512], fp32, name=f"pj{hp}_{nb}",
                                     tag="m")
                    for j, k in enumerate(range(0, NKT, 2)):
                        nc.tensor.matmul(
                            ps, lhsT=w4[:, k:k + 2, 0, hsl],
                            rhs=x4[:, k:k + 2, 0, tb],
                            start=(j == 0), stop=False, perf_mode=DR)
                    for k in range(NKT):
                        nc.tensor.matmul(
                            ps, lhsT=w4s[:, k, :, hsl],
                            rhs=x4[:, k, :, tb],
                            start=False, stop=(k == NKT - 1), perf_mode=DR)
                    nc.vector.tensor_scalar(
                        dst[:, tb], ps, 1.0 / 64.0, bsb[:, hp:hp + 1],
                        mybir.AluOpType.mult, mybir.AluOpType.add)
                closures.append(cl)
        return qt, kt, closures

    # ---- V for ALL heads: per key-tile [128 tok, 6*65] fp16, ones col per
    # head for the softmax denominator ----
    v_sb = [vpool.tile([128, 65 * 2 * HP], fp16, name=f"v_{si}", tag=f"v{si}")
            for si in range(NS)]

    v_done = [False] * NS

    def build_v():
        closures = []
        for si in range(NS):
            def cl(si=si, vt=v_sb[si]):
                v_done[si] = True
                ssl = slice(si * 128, (si + 1) * 128)
                nc.vector.memset(vt, 1.0)
                ps = psum_m.tile([128, EL], fp32, name=f"pv{si}", tag="m")
                for j, k in enumerate(range(0, NKT, 2)):
                    nc.tensor.matmul(
                        ps, lhsT=x4[:, k:k + 2, 0, ssl],
                        rhs=w4v[:, k:k + 2, 0, :],
                        start=(j == 0), stop=False, perf_mode=DR)
                for k in range(NKT):
                    nc.tensor.matmul(
                        ps, lhsT=x4[:, k, :, ssl],
                        rhs=w4vs[:, k, :, :],
                        start=False, stop=(k == NKT - 1), perf_mode=DR)
                nc.vector.scalar_tensor_tensor(
                    vt.rearrange("p (h w) -> p h w", w=65)[:, :, 0:64],
                    ps.rearrange("p (h w) -> p h w", w=64),
                    1.0 / 64.0,
                    bv_sb.rearrange("p (h w) -> p h w", w=64),
                    mybir.AluOpType.mult, mybir.AluOpType.add)
            closures.append(cl)
        return closures

    # ---- out-projection: Y[si] = ctx @ Wo_loc^T (fp16), fp16 DMA out ----
    def build_y(si, gp=False):
        # two closures (one per 384-col chunk) so the drip stays fine-grained
        cell = {}

        def chunk(nh, si=si):
            if nh == 0:
                cell["y"] = ypool.tile([128, Ecfg], fp16, name=f"y{si}",
                                       tag="y")
            y_sb = cell["y"]
            # tail closures alternate into the ctx bank (free once the
            # final normalize has drained) to overlap drains with matmuls
            p = psum_c if (gp and nh == 1) else psum_m
            yps = p.tile([128, 384], fp32, name=f"yp{si}_{nh}",
                         tag="c" if p is psum_c else "m")
            for hp in range(HP):
                nc.tensor.matmul(
                    yps, lhsT=ctxT[hp][:, si * 128:(si + 1) * 128],
                    rhs=wo_sb[hp][:, nh * 384:(nh + 1) * 384],
                    start=(hp == 0), stop=(hp == HP - 1))
            nc.vector.tensor_copy(y_sb[:, nh * 384:(nh + 1) * 384], yps)
            if nh == 1:
                nc.sync.dma_start(out=Y[si * 128:(si + 1) * 128, :], in_=y_sb)
        return [lambda nh=nh: chunk(nh) for nh in range(2)]

    y_by_qb = {qb: [cl for si in range(qb * (NS // NQB),
                                       (qb + 1) * (NS // NQB))
                    for cl in build_y(si, gp=(qb == NQB - 1))]
               for qb in range(NQB)}

    # ---- trailing ctx: natural [q, 65] orientation, accumulated per
    # (head, q-block) in a time-split PSUM bank; on the head's last k-tile,
    # normalize (DVE) and queue the PE transpose into the drip stream so
    # the PE never waits on the normalize chain ----
    def build_transp(hp_u, qb_u, hh, ctxn):
        def cl():
            tp = psum_m.tile([64, 512], fp16, name=f"tp{hp_u}_{qb_u}_{hh}",
                             tag="m")
            for qt in range(4):
                nc.tensor.matmul(
                    tp[:, qt * 128:qt * 128 + 128],
                    lhsT=ctxn[:, qt * 64:qt * 64 + 64],
                    rhs=idt_sb, is_transpose=True)
            nc.vector.tensor_copy(
                ctxT[hp_u][hh * 64:hh * 64 + 64,
                           qb_u * 512:qb_u * 512 + 512], tp)
        return cl

    # PSUM start=True lazily zeroes the whole 2KB bank, so the four 65-col
    # ctx regions sharing a bank must accumulate strictly one-after-another
    # (qt-major): a region's 16-step accumulation may not interleave with a
    # sibling region's start.
    def ctx_item(u, hh, qt, k):
        h6 = 2 * u["hp"] + hh
        if qt == 0 and k == 0:
            u["cps"] = psum_c.tile([128, 4 * 65], fp32,
                                   name=f"cps{u['hp']}_{u['qb']}_{hh}",
                                   tag="c")
        cps = u["cps"]
        r, j = divmod(hh * NS + k, ROT)
        slab = u["slabs"][r]
        nc.tensor.matmul(
            cps[:, qt * 65:qt * 65 + 65],
            lhsT=slab[:, j * 512 + qt * 128: j * 512 + qt * 128 + 128],
            rhs=v_sb[k][:, h6 * 65:h6 * 65 + 65],
            start=(k == 0), stop=(k == NS - 1))
        if qt == 3 and k == NS - 1:
            hp_u, qb_u = u["hp"], u["qb"]
            c3 = cps.rearrange("p (qt c) -> p qt c", c=65)
            rc = npool.tile([128, 4], fp32, name=f"rc{hp_u}_{qb_u}_{hh}",
                            tag="rc")
            nc.vector.reciprocal(rc, c3[:, :, 64:65])
            ctxn = npool.tile([128, 256], fp16, name=f"cn{hp_u}_{qb_u}_{hh}",
                              tag="cn")
            for q2 in range(4):
                nc.vector.tensor_scalar_mul(
                    ctxn[:, q2 * 64:q2 * 64 + 64], c3[:, q2, 0:64],
                    rc[:, q2:q2 + 1])
            pending.appendleft(build_transp(hp_u, qb_u, hh, ctxn))

    # global rotation history for the lag-3 ctx trail; trail items are
    # single matmuls (unit, hh, qt, k) gated on their slab's exp rotation
    LAG = 3
    hist = []            # cumulative combos available after each global rot
    trail_q = collections.deque()   # (unit, hh, qt, k, gate)
    slow = collections.deque()      # heavy closures, paced 1 per 3 rotations

    def trail_to(target, cap=24):
        # pause before a fresh PSUM bank (hh, qt=0, k=0) so the previous
        # half-unit's normalize has a rotation of shadow
        emitted = 0
        while trail_q and emitted < cap:
            u, hh, qt, k, gate = trail_q[0]
            if gate >= target or not v_done[k]:
                break
            if emitted and qt == 0 and k == 0:
                break
            trail_q.popleft()
            ctx_item(u, hh, qt, k)
            emitted += 1

    cur = build_proj(0)
    vcl = build_v()
    qt_dbg = None
    for hp in range(HP):
        qt, kt, closures = cur
        if hp == 0:
            qt_dbg = (qt, kt)
        if hp == 0:
            # minimal prefix so unit (0,0) can start; keys first, V paced
            # to arrive before the (deferred) ctx trail consumes it.
            closures[0]()
            closures[NQB]()
            krest = closures[NQB + 1:2 * NQB]
            qrest = closures[1:NQB]
            pend0 = (krest[:2] + qrest[:1] + krest[2:] + qrest[1:] + vcl)
            pending.extend(pend0)
        cur = build_proj(hp + 1) if hp + 1 < HP else None
        if cur is not None:
            slow.extend(cur[2])

        if hp == 1 and "DQT" in io:
            nc.sync.dma_start(out=io["DQT"], in_=qt_dbg[0])
            nc.sync.dma_start(out=io["DKT"], in_=qt_dbg[1])
            nc.sync.dma_start(out=io["DV0"], in_=v_sb[0])
        for qb in range(NQB):
            qsl = slice(qb * 512, qb * 512 + 512)
            unit = dict(hp=hp, qb=qb, slabs=[], cps=None)
            first_unit = (hp == 0 and qb == 0)
            base = NCOMBO * (hp * NQB + qb)
            for hh_ in range(2):
                for qt_ in range(4):
                    for k_ in range(NS):
                        gate = base + hh_ * NS + (k_ if qt_ == 0 else NS - 1)
                        trail_q.append((unit, hh_, qt_, k_, gate))
            for r in range(NROT):
                if hp == HP - 1 and qb >= 1 and r == 4:
                    pending.extend(y_by_qb[qb - 1])
                cis = range(ROT * r, min(ROT * r + ROT, NCOMBO))
                n = len(cis)
                g = psum_g.tile([128, 512 * ROT], fp32,
                                name=f"g{hp}_{qb}_{r}", tag="g")
                for j, ci in enumerate(cis):
                    hh, k = divmod(ci, NS)
                    nc.tensor.matmul(
                        g[:, j * 512:(j + 1) * 512],
                        lhsT=kt[hh * 64:(hh + 1) * 64, k * 128:(k + 1) * 128],
                        rhs=qt[hh * 64:(hh + 1) * 64, qsl],
                        start=True, stop=True)
                slab = spool.tile([128, 512 * ROT], fp16,
                                  name=f"s{hp}_{qb}_{r}", tag="slab")
                if r in SCH:
                    nc.vector.tensor_scalar(
                        slab[:, :n * 512].bitcast(i16), g[:, :n * 512],
                        SCH_A, SCH_B, mybir.AluOpType.mult,
                        mybir.AluOpType.add)
                else:
                    nc.scalar.activation(slab[:, :n * 512], g[:, :n * 512],
                                         Exp, scale=0.125)
                unit["slabs"].append(slab)
                hist.append((hist[-1] if hist else 0) + n)
                gi = len(hist) - 1
                if gi >= 2 * LAG:
                    trail_to(hist[gi - LAG])
                if slow and gi % 3 == 1:
                    pending.append(slow.popleft())
                drip(3 if first_unit else 1)

    # drain the remaining ctx matmuls, dripping in the leftover closures
    nt = 0
    while trail_q:
        u, hh, qt, k, gate = trail_q.popleft()
        ctx_item(u, hh, qt, k)
        nt += 1
        if nt % 8 == 0:
            drip(1)
    pending.extend(slow)
    slow.clear()
    pending.extend(y_by_qb[NQB - 1])
    drip(len(pending))
    if "DCT" in io:
        for hp in range(HP):
            nc.sync.dma_start(
                out=io["DCT"][hp * 128:(hp + 1) * 128, :], in_=ctxT[hp])


def _build():
    import contextlib
    import concourse.mybir as mybir
    import concourse.tile as tile
    from concourse import bacc

    fp32, fp16 = mybir.dt.float32, mybir.dt.float16
    fp8 = mybir.dt.float8e4
    Scfg, Ecfg, EL, NKT, HP = 2048, 768, 384, 6, 3

    nc = bacc.Bacc("TRN2", target_bir_lowering=False, debug=False,
                   num_devices=NCORES)
    io = {
        "X8": nc.dram_tensor("X8", [128, NKT * 2 * Scfg], fp8,
                             kind="ExternalInput").ap(),
        "W8Q": nc.dram_tensor("W8Q", [128, NKT * 2 * EL], fp8,
                              kind="ExternalInput").ap(),
        "W8QS": nc.dram_tensor("W8QS", [128, NKT * 2 * EL], fp8,
                               kind="ExternalInput").ap(),
        "W8K": nc.dram_tensor("W8K", [128, NKT * 2 * EL], fp8,
                              kind="ExternalInput").ap(),
        "W8KS": nc.dram_tensor("W8KS", [128, NKT * 2 * EL], fp8,
                               kind="ExternalInput").ap(),
        "W8V": nc.dram_tensor("W8V", [128, NKT * 2 * EL], fp8,
                              kind="ExternalInput").ap(),
        "W8VS": nc.dram_tensor("W8VS", [128, NKT * 2 * EL], fp8,
                               kind="ExternalInput").ap(),
        "WOT": nc.dram_tensor("WOT", [EL, Ecfg], fp16,
                              kind="ExternalInput").ap(),
        "BQ": nc.dram_tensor("BQ", [128, HP], fp32, kind="ExternalInput").ap(),
        "BK": nc.dram_tensor("BK", [128, HP], fp32, kind="ExternalInput").ap(),
        "BV": nc.dram_tensor("BV", [128, EL], fp32, kind="ExternalInput").ap(),
        "IDT": nc.dram_tensor("IDT", [128, 128], fp16,
                              kind="ExternalInput").ap(),
        "Y": nc.dram_tensor("Y", [Scfg, Ecfg], fp16,
                            kind="ExternalOutput").ap(),
    }
    with tile.TileContext(nc) as tc:
        with contextlib.ExitStack() as ctx:
            _emit(nc, tc, ctx, io)
    nc.compile()
    return nc


def _get_program():
    if "full" not in _cache:
        _cache["full"] = _build()
    return _cache["full"]


def _hilo_pack(a):
    """[128, k, cols] fp32 -> [128, k, 2, cols] (hi, lo) fp8 pair + swapped."""
    import ml_dtypes
    e4 = ml_dtypes.float8_e4m3
    hi = a.astype(e4)
    lo = (a - hi.astype(np.float32)).astype(e4)
    norm = np.stack([hi, lo], axis=2)
    swap = np.stack([lo, hi], axis=2)
    return norm, swap


def _k_major(a, ncols):
    """[rows=k*128, ncols] -> [128, k, ncols] (contraction-tile-major)."""
    k = a.shape[0] // 128
    return np.ascontiguousarray(a.reshape(k, 128, ncols).transpose(1, 0, 2))


def _half_inputs(half, Wq, bq, Wk, bk, Wv, bv, Wo):
    import ml_dtypes
    f16 = np.float16
    e0 = 384 * half
    ecols = slice(e0, e0 + 384)
    out = {}
    for nm, W in (("Q", Wq), ("K", Wk), ("V", Wv)):
        # x64 pre-scale keeps the lo residual above e4m3's subnormal floor;
        # the PSUM drain multiplies by 1/64.
        wt = _k_major(np.ascontiguousarray(W[ecols, :].T) * 64.0, 384)
        norm, swap = _hilo_pack(wt)
        out[f"W8{nm}"] = norm.reshape(128, -1)
        out[f"W8{nm}S"] = swap.reshape(128, -1)
    out["WOT"] = np.ascontiguousarray(Wo[:, ecols].T).astype(f16)
    out["BQ"] = np.ascontiguousarray(bq[ecols].reshape(3, 128).T).astype(np.float32)
    out["BK"] = np.ascontiguousarray(bk[ecols].reshape(3, 128).T).astype(np.float32)
    out["BV"] = np.ascontiguousarray(
        np.broadcast_to(bv[ecols], (128, 384))).astype(np.float32)
    out["IDT"] = np.eye(128, dtype=f16)
    return out


def kernel(X, Wq, bq, Wk, bk, Wv, bv, Wo, bo):
    from concourse.bass_utils import run_bass_kernel_spmd

    X, Wq, bq, Wk, bk, Wv, bv, Wo, bo = [
        np.asarray(a, dtype=np.float32)
        for a in (X, Wq, bq, Wk, bk, Wv, bv, Wo, bo)
    ]
    nc = _get_program()
    halves = [_half_inputs(h, Wq, bq, Wk, bk, Wv, bv, Wo) for h in range(2)]
    x8s = []
    for b in range(B):
        xt = _k_major(np.ascontiguousarray(X[b].T), 2048)
        norm, _ = _hilo_pack(xt)
        x8s.append(norm.reshape(128, -1))
    in_maps = [dict(halves[c % 2], X8=x8s[c // 2]) for c in range(NCORES)]
    res = run_bass_kernel_spmd(nc, in_maps, list(range(NCORES)))
    out = np.empty((B, S, E), np.float32)
    for b in range(B):
        out[b] = (res.results[2 * b]["Y"].astype(np.float32)
                  + res.results[2 * b + 1]["Y"].astype(np.float32)
                  + bo[None, :])
    return out


# revision 58
# speedup vs baseline: 1.1712x; 1.0217x over previous
"""Multi-head self-attention Trainium2 kernel (8 NeuronCores).

Sharding: 8 cores = 4 batches x 2 head-halves (6 heads each). Per core:
QKV projections run as fp8(e4m3) hi+lo DoubleRow matmuls (host splits X
and W into hi/lo fp8 pairs; the lo*lo term is dropped), scores S^T = K.Q^T
in fp16 per (head, key-tile, q-block), P = exp(S/8) on ScalarE straight
out of 3-bank PSUM groups, ctx in the natural [q, d] orientation
(lhsT = P^T slab slices, rhs = V with a ones column for the softmax
denominator) accumulated per (head, q-block) in a time-split PSUM bank,
normalization + PE transpose into ctx^T, and the out-projection
Y_partial = ctx @ Wo[:, cols]^T in fp16. The host sums the two per-batch
partials and adds the output bias.

Scheduling: each unit's ctx matmuls trail one unit behind the score/exp
stream (head-major combo order so the shared ctx PSUM bank is time-split
between the two heads); projections for the next head pair, V tiles,
transposes and out-projection slices drip into the rotation stream.
"""

import numpy as np

B, S, E, H, D = 4, 2048, 768, 12, 64
NCORES = 8

_cache = {}


def _emit(nc, tc, ctx, io):
    import concourse.mybir as mybir

    fp32 = mybir.dt.float32
    fp16 = mybir.dt.float16
    fp8 = mybir.dt.float8e4
    i16 = mybir.dt.int16
    Exp = mybir.ActivationFunctionType.Exp
    DR = mybir.MatmulPerfMode.DoubleRow

    Scfg, Ecfg, EL = 2048, 768, 384
    NKT = Ecfg // 128           # contraction tiles over embed dim (6)
    NS = Scfg // 128            # key tiles (16)
    NQB = Scfg // 512           # q-blocks (4)
    HP = EL // 128              # head pairs (3)
    ROT = 3                     # exp group size in PSUM banks
    NCOMBO = 2 * NS             # (head, k-tile) combos per unit (32)
    NROT = (NCOMBO + ROT - 1) // ROT  # 11
    # Rotations offloaded to a DVE exp approximation: two phase-shifted
    # int16-bitcast Schraudolph evaluations (B-phase magnitude-compensated
    # by a 1/sqrt(2)-scaled V copy) average to ~0.8% max element error.
    SCH = frozenset()
    SCH_A = float(1024.0 * np.log2(np.e) / 8.0)
    SCH_BA = float(1024.0 * (15.0 - 0.054) + 0.5 - 1024.0)
    SCH_BB = SCH_BA + 512.0

    X8, W8Q, W8QS, W8K, W8KS, W8V, W8VS, WOT, BQ, BK, BV, IDT, Y = (
        io["X8"], io["W8Q"], io["W8QS"], io["W8K"], io["W8KS"], io["W8V"],
        io["W8VS"], io["WOT"], io["BQ"], io["BK"], io["BV"], io["IDT"],
        io["Y"],
    )

    consts = ctx.enter_context(tc.tile_pool(name="consts", bufs=1))
    wpool = ctx.enter_context(tc.tile_pool(name="wpool", bufs=1))
    xpool = ctx.enter_context(tc.tile_pool(name="xpool", bufs=1))
    qkpool = ctx.enter_context(tc.tile_pool(name="qkpool", bufs=4))
    vpool = ctx.enter_context(tc.tile_pool(name="vpool", bufs=1))
    spool = ctx.enter_context(tc.tile_pool(name="spool", bufs=27))
    cpool = ctx.enter_context(tc.tile_pool(name="cpool", bufs=1))
    npool = ctx.enter_context(tc.tile_pool(name="npool", bufs=4))
    ypool = ctx.enter_context(tc.tile_pool(name="ypool", bufs=2))
    psum_g = ctx.enter_context(tc.tile_pool(name="psum_g", bufs=2, space="PSUM"))
    psum_c = ctx.enter_context(tc.tile_pool(name="psum_c", bufs=1, space="PSUM"))
    psum_m = ctx.enter_context(tc.tile_pool(name="psum_m", bufs=1, space="PSUM"))

    # ---- weights / constants to SBUF (critical-path tensors first) ----
    # DMA issue order = first-projection critical path: weights+chunk0,
    # then the (tiny) bias the first drain needs, then the cross weights
    w8q_sb = wpool.tile([128, NKT * 2 * EL], fp8, name="w8q_sb")
    nc.sync.dma_start(out=w8q_sb, in_=W8Q)
    w8k_sb = wpool.tile([128, NKT * 2 * EL], fp8, name="w8k_sb")
    nc.sync.dma_start(out=w8k_sb, in_=W8K)
    x8_sb = xpool.tile([128, NKT * 2 * Scfg], fp8, name="x8_sb")
    x4s = x8_sb.rearrange("p (k h t) -> p k h t", k=NKT, h=2)
    x4d = X8.rearrange("p (k h t) -> p k h t", k=NKT, h=2)

    def xchunk(tc_):
        tsl = slice(tc_ * 512, (tc_ + 1) * 512)
        nc.sync.dma_start(out=x4s[:, :, :, tsl], in_=x4d[:, :, :, tsl])

    xchunk(0)
    bq_sb = consts.tile([128, HP], fp32, name="bq_sb")
    nc.sync.dma_start(out=bq_sb, in_=BQ)
    bk_sb = consts.tile([128, HP], fp32, name="bk_sb")
    nc.sync.dma_start(out=bk_sb, in_=BK)
    w8qs_sb = wpool.tile([128, NKT * 2 * EL], fp8, name="w8qs_sb")
    nc.sync.dma_start(out=w8qs_sb, in_=W8QS)
    w8ks_sb = wpool.tile([128, NKT * 2 * EL], fp8, name="w8ks_sb")
    nc.sync.dma_start(out=w8ks_sb, in_=W8KS)
    for tc_ in range(1, 4):
        xchunk(tc_)
    bv_sb = consts.tile([128, EL], fp32, name="bv_sb")
    nc.sync.dma_start(out=bv_sb, in_=BV)
    idt_sb = consts.tile([128, 128], fp16, name="idt_sb")
    nc.sync.dma_start(out=idt_sb, in_=IDT)
    w8v_sb = wpool.tile([128, NKT * 2 * EL], fp8, name="w8v_sb")
    nc.sync.dma_start(out=w8v_sb, in_=W8V)
    w8vs_sb = wpool.tile([128, NKT * 2 * EL], fp8, name="w8vs_sb")
    nc.sync.dma_start(out=w8vs_sb, in_=W8VS)

    wo_sb = []
    for hp in range(HP):
        t = wpool.tile([128, Ecfg], fp16, name=f"wo{hp}_sb", tag=f"wo{hp}")
        nc.sync.dma_start(out=t, in_=WOT[hp * 128:(hp + 1) * 128, :])
        wo_sb.append(t)

    # 4D views: [p, ktile, hi/lo, cols]
    x4 = x8_sb.rearrange("p (k h t) -> p k h t", k=NKT, h=2)
    w4q = w8q_sb.rearrange("p (k h c) -> p k h c", k=NKT, h=2)
    w4qs = w8qs_sb.rearrange("p (k h c) -> p k h c", k=NKT, h=2)
    w4k = w8k_sb.rearrange("p (k h c) -> p k h c", k=NKT, h=2)
    w4ks = w8ks_sb.rearrange("p (k h c) -> p k h c", k=NKT, h=2)
    w4v = w8v_sb.rearrange("p (k h c) -> p k h c", k=NKT, h=2)
    w4vs = w8vs_sb.rearrange("p (k h c) -> p k h c", k=NKT, h=2)

    ctxT = []
    for hp in range(HP):
        t = cpool.tile([128, Scfg], fp16, name=f"ctxT{hp}", tag=f"ctx{hp}")
        ctxT.append(t)

    import collections
    pending = collections.deque()
    chain = [None]   # a closure that MUST be the next pop (its PSUM bank is
                     # mid-accumulation — nothing may allocate in between)

    def drip(n=1):
        for _ in range(n):
            if chain[0] is not None:
                cl, chain[0] = chain[0], None
            elif pending:
                cl = pending.popleft()
            else:
                return
            cl()

    # ---- fp8 hi/lo DoubleRow projection closures ----
    def build_proj(hp):
        qt = qkpool.tile([128, Scfg], fp16, name=f"qt{hp}", tag="qt")
        kt = qkpool.tile([128, Scfg], fp16, name=f"kt{hp}", tag="kt")
        hsl = slice(hp * 128, (hp + 1) * 128)
        closures = []
        for dst, w4, w4s, bsb in ((qt, w4q, w4qs, bq_sb), (kt, w4k, w4ks, bk_sb)):
            for nb in range(NQB):
                def cl(dst=dst, w4=w4, w4s=w4s, bsb=bsb, nb=nb, hp=hp, hsl=hsl):
                    tb = slice(nb * 512, (nb + 1) * 512)
                    ps = psum_m.tile([128, 512], fp32, name=f"pj{hp}_{nb}",
                                     tag="m")
                    for j, k in enumerate(range(0, NKT, 2)):
                        nc.tensor.matmul(
                            ps, lhsT=w4[:, k:k + 2, 0, hsl],
                            rhs=x4[:, k:k + 2, 0, tb],
                            start=(j == 0), stop=False, perf_mode=DR)

                    def cl2(ps=ps):
                        for k in range(NKT):
                            nc.tensor.matmul(
                                ps, lhsT=w4s[:, k, :, hsl],
                                rhs=x4[:, k, :, tb],
                                start=False, stop=(k == NKT - 1), perf_mode=DR)
                        nc.vector.tensor_scalar(
                            dst[:, tb], ps, 1.0 / 64.0, bsb[:, hp:hp + 1],
                            mybir.AluOpType.mult, mybir.AluOpType.add)
                    chain[0] = cl2
                closures.append(cl)
        return qt, kt, closures

    # ---- V for ALL heads: per key-tile [128 tok, 6*65] fp16, ones col per
    # head for the softmax denominator; v2 = V/sqrt(2) feeds the B-phase of
    # the DVE exp approximation ----
    v_sb = [vpool.tile([128, 65 * 2 * HP], fp16, name=f"v_{si}", tag=f"v{si}")
            for si in range(NS)]
    sch_k = sorted({ci % NS for r in SCH
                    for ci in range(ROT * r, min(ROT * r + ROT, NCOMBO))})
    v2_sb = {si: vpool.tile([128, 65 * 2 * HP], fp16, name=f"v2_{si}",
                            tag=f"v2{si}")
             for si in sch_k}

    v_done = [False] * NS

    def build_v():
        closures = []
        for si in range(NS):
            def cl(si=si, vt=v_sb[si]):
                ssl = slice(si * 128, (si + 1) * 128)
                nc.vector.memset(vt, 1.0)
                ps = psum_m.tile([128, EL], fp32, name=f"pv{si}", tag="m")
                for j, k in enumerate(range(0, NKT, 2)):
                    nc.tensor.matmul(
                        ps, lhsT=x4[:, k:k + 2, 0, ssl],
                        rhs=w4v[:, k:k + 2, 0, :],
                        start=(j == 0), stop=False, perf_mode=DR)

                def cl2(si=si, vt=vt, ps=ps):
                    for k in range(NKT):
                        nc.tensor.matmul(
                            ps, lhsT=x4[:, k, :, ssl],
                            rhs=w4vs[:, k, :, :],
                            start=False, stop=(k == NKT - 1), perf_mode=DR)
                    nc.vector.scalar_tensor_tensor(
                        vt.rearrange("p (h w) -> p h w", w=65)[:, :, 0:64],
                        ps.rearrange("p (h w) -> p h w", w=64),
                        1.0 / 64.0,
                        bv_sb.rearrange("p (h w) -> p h w", w=64),
                        mybir.AluOpType.mult, mybir.AluOpType.add)
                    if si in v2_sb:
                        nc.vector.tensor_scalar_mul(v2_sb[si], vt,
                                                    float(2.0 ** -0.5))
                    v_done[si] = True
                chain[0] = cl2
            closures.append(cl)
        return closures

    # ---- out-projection: Y[si] = ctx @ Wo_loc^T (fp16), fp16 DMA out ----
    def build_y(si, gp=False):
        # two closures (one per 384-col chunk) so the drip stays fine-grained
        cell = {}

        def chunk(nh, si=si):
            if nh == 0:
                cell["y"] = ypool.tile([128, Ecfg], fp16, name=f"y{si}",
                                       tag="y")
            y_sb = cell["y"]
            # tail closures alternate into the ctx bank (free once the
            # final normalize has drained) to overlap drains with matmuls
            p = psum_c if (gp and nh == 1) else psum_m
            yps = p.tile([128, 384], fp32, name=f"yp{si}_{nh}",
                         tag="c" if p is psum_c else "m")
            for hp in range(HP):
                nc.tensor.matmul(
                    yps, lhsT=ctxT[hp][:, si * 128:(si + 1) * 128],
                    rhs=wo_sb[hp][:, nh * 384:(nh + 1) * 384],
                    start=(hp == 0), stop=(hp == HP - 1))
            if gp and nh == 0:
                # tail: ACT is idle — split the drains across engines
                nc.scalar.copy(y_sb[:, nh * 384:(nh + 1) * 384], yps)
            else:
                nc.vector.tensor_copy(y_sb[:, nh * 384:(nh + 1) * 384], yps)
            if nh == 1:
                nc.sync.dma_start(out=Y[si * 128:(si + 1) * 128, :], in_=y_sb)
        return [lambda nh=nh: chunk(nh) for nh in range(2)]

    y_by_qb = {qb: [cl for si in range(qb * (NS // NQB),
                                       (qb + 1) * (NS // NQB))
                    for cl in build_y(si, gp=(qb == NQB - 1))]
               for qb in range(NQB)}

    # ---- trailing ctx: natural [q, 65] orientation, accumulated per
    # (head, q-block) in a time-split PSUM bank; on the head's last k-tile,
    # normalize (DVE) and queue the PE transpose into the drip stream so
    # the PE never waits on the normalize chain ----
    def build_transp(hp_u, qb_u, hh, ctxn):
        def cl():
            tp = psum_m.tile([64, 512], fp16, name=f"tp{hp_u}_{qb_u}_{hh}",
                             tag="m")
            for qt in range(4):
                nc.tensor.matmul(
                    tp[:, qt * 128:qt * 128 + 128],
                    lhsT=ctxn[:, qt * 64:qt * 64 + 64],
                    rhs=idt_sb, is_transpose=True)
            nc.vector.tensor_copy(
                ctxT[hp_u][hh * 64:hh * 64 + 64,
                           qb_u * 512:qb_u * 512 + 512], tp)
        return cl

    # PSUM start=True lazily zeroes the whole 2KB bank, so the four 65-col
    # ctx regions sharing a bank must accumulate strictly one-after-another
    # (qt-major): a region's 16-step accumulation may not interleave with a
    # sibling region's start.
    def ctx_item(u, hh, qt, k):
        h6 = 2 * u["hp"] + hh
        if qt == 0 and k == 0:
            u["cps"] = psum_c.tile([128, 4 * 65], fp32,
                                   name=f"cps{u['hp']}_{u['qb']}_{hh}",
                                   tag="c")
        cps = u["cps"]
        r, j = divmod(hh * NS + k, ROT)
        slab = u["slabs"][r]
        csl = slice(qt * 65, qt * 65 + 65)
        ssl = slice(j * 512 + qt * 128, j * 512 + qt * 128 + 128)
        if isinstance(slab, tuple):
            sa, sb = slab
            nc.tensor.matmul(cps[:, csl], lhsT=sa[:, ssl],
                             rhs=v_sb[k][:, h6 * 65:h6 * 65 + 65],
                             start=(k == 0), stop=False)
            nc.tensor.matmul(cps[:, csl], lhsT=sb[:, ssl],
                             rhs=v2_sb[k][:, h6 * 65:h6 * 65 + 65],
                             start=False, stop=(k == NS - 1))
        else:
            nc.tensor.matmul(
                cps[:, csl], lhsT=slab[:, ssl],
                rhs=v_sb[k][:, h6 * 65:h6 * 65 + 65],
                start=(k == 0), stop=(k == NS - 1))
        if qt == 3 and k == NS - 1:
            hp_u, qb_u = u["hp"], u["qb"]
            c3 = cps.rearrange("p (qt c) -> p qt c", c=65)
            rc = npool.tile([128, 4], fp32, name=f"rc{hp_u}_{qb_u}_{hh}",
                            tag="rc")
            nc.vector.reciprocal(rc, c3[:, :, 64:65])
            ctxn = npool.tile([128, 256], fp16, name=f"cn{hp_u}_{qb_u}_{hh}",
                              tag="cn")
            for q2 in range(4):
                nc.vector.tensor_scalar_mul(
                    ctxn[:, q2 * 64:q2 * 64 + 64], c3[:, q2, 0:64],
                    rc[:, q2:q2 + 1])
            pending.appendleft(build_transp(hp_u, qb_u, hh, ctxn))

    # global rotation history for the lag-3 ctx trail; trail items are
    # single matmuls (unit, hh, qt, k) gated on their slab's exp rotation
    LAG = 3
    hist = []            # cumulative combos available after each global rot
    trail_q = collections.deque()   # (unit, hh, qt, k, gate)
    slow = collections.deque()      # heavy closures, paced 1 per 3 rotations

    def trail_to(target, cap=24):
        # pause before a fresh PSUM bank (hh, qt=0, k=0) so the previous
        # half-unit's normalize has a rotation of shadow
        emitted = 0
        while trail_q and emitted < cap:
            u, hh, qt, k, gate = trail_q[0]
            if gate >= target or not v_done[k]:
                break
            if emitted and qt == 0 and k == 0:
                break
            trail_q.popleft()
            ctx_item(u, hh, qt, k)
            emitted += 1

    cur = build_proj(0)
    vcl = build_v()
    qt_dbg = None
    for hp in range(HP):
        qt, kt, closures = cur
        if hp == 0:
            qt_dbg = (qt, kt)
        if hp == 0:
            # minimal prefix so unit (0,0) can start; keys first, V paced
            # to arrive before the (deferred) ctx trail consumes it.
            for cl in (closures[0], closures[NQB]):
                cl()
                if chain[0] is not None:
                    c2, chain[0] = chain[0], None
                    c2()
            krest = closures[NQB + 1:2 * NQB]
            qrest = closures[1:NQB]
            pend0 = (krest[:2] + qrest[:1] + krest[2:] + qrest[1:] + vcl)
            pending.extend(pend0)
        cur = build_proj(hp + 1) if hp + 1 < HP else None
        if cur is not None:
            pending.extend(cur[2])

        if hp == 1 and "DQT" in io:
            nc.sync.dma_start(out=io["DQT"], in_=qt_dbg[0])
            nc.sync.dma_start(out=io["DKT"], in_=qt_dbg[1])
            nc.sync.dma_start(out=io["DV0"], in_=v_sb[0])
        for qb in range(NQB):
            qsl = slice(qb * 512, qb * 512 + 512)
            unit = dict(hp=hp, qb=qb, slabs=[], cps=None)
            first_unit = (hp == 0 and qb == 0)
            base = NCOMBO * (hp * NQB + qb)
            for hh_ in range(2):
                for qt_ in range(4):
                    for k_ in range(NS):
                        gate = base + hh_ * NS + (k_ if qt_ == 0 else NS - 1)
                        trail_q.append((unit, hh_, qt_, k_, gate))
            for r in range(NROT):
                if hp == HP - 1 and qb >= 1 and r == 6:
                    pending.extend(y_by_qb[qb - 1])
                cis = range(ROT * r, min(ROT * r + ROT, NCOMBO))
                n = len(cis)
                g = psum_g.tile([128, 512 * ROT], fp32,
                                name=f"g{hp}_{qb}_{r}", tag="g")
                for j, ci in enumerate(cis):
                    hh, k = divmod(ci, NS)
                    nc.tensor.matmul(
                        g[:, j * 512:(j + 1) * 512],
                        lhsT=kt[hh * 64:(hh + 1) * 64, k * 128:(k + 1) * 128],
                        rhs=qt[hh * 64:(hh + 1) * 64, qsl],
                        start=True, stop=True)
                drip(1)
                slab = spool.tile([128, 512 * ROT], fp16,
                                  name=f"s{hp}_{qb}_{r}", tag="slab")
                if r in SCH:
                    slab_b = spool.tile([128, 512 * ROT], fp16,
                                        name=f"sb{hp}_{qb}_{r}", tag="slab")
                    nc.vector.tensor_scalar(
                        slab[:, :n * 512].bitcast(i16), g[:, :n * 512],
                        SCH_A, SCH_BA, mybir.AluOpType.mult,
                        mybir.AluOpType.add)
                    nc.vector.tensor_scalar(
                        slab_b[:, :n * 512].bitcast(i16), g[:, :n * 512],
                        SCH_A, SCH_BB, mybir.AluOpType.mult,
                        mybir.AluOpType.add)
                    unit["slabs"].append((slab, slab_b))
                else:
                    nc.scalar.activation(slab[:, :n * 512], g[:, :n * 512],
                                         Exp, scale=0.125)
                    unit["slabs"].append(slab)
                hist.append((hist[-1] if hist else 0) + n)
                gi = len(hist) - 1
                if gi >= 2 * LAG:
                    trail_to(hist[gi - LAG])
                drip(2 if first_unit else 0)

    # drain the remaining ctx matmuls, dripping in the leftover closures
    nt = 0
    while trail_q:
        u, hh, qt, k, gate = trail_q.popleft()
        ctx_item(u, hh, qt, k)
        nt += 1
        if nt % 4 == 0:
            drip(1)
    pending.extend(y_by_qb[NQB - 1])
    drip(10000)
    if "DCT" in io:
        for hp in range(HP):
            nc.sync.dma_start(
                out=io["DCT"][hp * 128:(hp + 1) * 128, :], in_=ctxT[hp])


def _build():
    import contextlib
    import concourse.mybir as mybir
    import concourse.tile as tile
    from concourse import bacc

    fp32, fp16 = mybir.dt.float32, mybir.dt.float16
    fp8 = mybir.dt.float8e4
    Scfg, Ecfg, EL, NKT, HP = 2048, 768, 384, 6, 3

    nc = bacc.Bacc("TRN2", target_bir_lowering=False, debug=False,
                   num_devices=NCORES)
    io = {
        "X8": nc.dram_tensor("X8", [128, NKT * 2 * Scfg], fp8,
                             kind="ExternalInput").ap(),
        "W8Q": nc.dram_tensor("W8Q", [128, NKT * 2 * EL], fp8,
                              kind="ExternalInput").ap(),
        "W8QS": nc.dram_tensor("W8QS", [128, NKT * 2 * EL], fp8,
                               kind="ExternalInput").ap(),
        "W8K": nc.dram_tensor("W8K", [128, NKT * 2 * EL], fp8,
                              kind="ExternalInput").ap(),
        "W8KS": nc.dram_tensor("W8KS", [128, NKT * 2 * EL], fp8,
                               kind="ExternalInput").ap(),
        "W8V": nc.dram_tensor("W8V", [128, NKT * 2 * EL], fp8,
                              kind="ExternalInput").ap(),
        "W8VS": nc.dram_tensor("W8VS", [128, NKT * 2 * EL], fp8,
                               kind="ExternalInput").ap(),
        "WOT": nc.dram_tensor("WOT", [EL, Ecfg], fp16,
                              kind="ExternalInput").ap(),
        "BQ": nc.dram_tensor("BQ", [128, HP], fp32, kind="ExternalInput").ap(),
        "BK": nc.dram_tensor("BK", [128, HP], fp32, kind="ExternalInput").ap(),
        "BV": nc.dram_tensor("BV", [128, EL], fp32, kind="ExternalInput").ap(),
        "IDT": nc.dram_tensor("IDT", [128, 128], fp16,
                              kind="ExternalInput").ap(),
        "Y": nc.dram_tensor("Y", [Scfg, Ecfg], fp16,
                            kind="ExternalOutput").ap(),
    }
    with tile.TileContext(nc) as tc:
        with contextlib.ExitStack() as ctx:
            _emit(nc, tc, ctx, io)
    nc.compile()
    return nc


def _get_program():
    if "full" not in _cache:
        _cache["full"] = _build()
    return _cache["full"]


def _hilo_pack(a):
    """[128, k, cols] fp32 -> [128, k, 2, cols] (hi, lo) fp8 pair + swapped."""
    import ml_dtypes
    e4 = ml_dtypes.float8_e4m3
    hi = a.astype(e4)
    lo = (a - hi.astype(np.float32)).astype(e4)
    norm = np.stack([hi, lo], axis=2)
    swap = np.stack([lo, hi], axis=2)
    return norm, swap


def _k_major(a, ncols):
    """[rows=k*128, ncols] -> [128, k, ncols] (contraction-tile-major)."""
    k = a.shape[0] // 128
    return np.ascontiguousarray(a.reshape(k, 128, ncols).transpose(1, 0, 2))


def _half_inputs(half, Wq, bq, Wk, bk, Wv, bv, Wo):
    import ml_dtypes
    f16 = np.float16
    e0 = 384 * half
    ecols = slice(e0, e0 + 384)
    out = {}
    for nm, W in (("Q", Wq), ("K", Wk), ("V", Wv)):
        # x64 pre-scale keeps the lo residual above e4m3's subnormal floor;
        # the PSUM drain multiplies by 1/64.
        wt = _k_major(np.ascontiguousarray(W[ecols, :].T) * 64.0, 384)
        norm, swap = _hilo_pack(wt)
        out[f"W8{nm}"] = norm.reshape(128, -1)
        out[f"W8{nm}S"] = swap.reshape(128, -1)
    out["WOT"] = np.ascontiguousarray(Wo[:, ecols].T).astype(f16)
    out["BQ"] = np.ascontiguousarray(bq[ecols].reshape(3, 128).T).astype(np.float32)
    out["BK"] = np.ascontiguousarray(bk[ecols].reshape(3, 128).T).astype(np.float32)
    out["BV"] = np.ascontiguousarray(
        np.broadcast_to(bv[ecols], (128, 384))).astype(np.float32)
    out["IDT"] = np.eye(128, dtype=f16)
    return out


def kernel(X, Wq, bq, Wk, bk, Wv, bv, Wo, bo):
    from concourse.bass_utils import run_bass_kernel_spmd

    X, Wq, bq, Wk, bk, Wv, bv, Wo, bo = [
        np.asarray(a, dtype=np.float32)
        for a in (X, Wq, bq, Wk, bk, Wv, bv, Wo, bo)
    ]
    nc = _get_program()
    halves = [_half_inputs(h, Wq, bq, Wk, bk, Wv, bv, Wo) for h in range(2)]
    x8s = []
    for b in range(B):
        xt = _k_major(np.ascontiguousarray(X[b].T), 2048)
        norm, _ = _hilo_pack(xt)
        x8s.append(norm.reshape(128, -1))
    in_maps = [dict(halves[c % 2], X8=x8s[c // 2]) for c in range(NCORES)]
    res = run_bass_kernel_spmd(nc, in_maps, list(range(NCORES)))
    out = np.empty((B, S, E), np.float32)
    for b in range(B):
        out[b] = (res.results[2 * b]["Y"].astype(np.float32)
                  + res.results[2 * b + 1]["Y"].astype(np.float32)
                  + bo[None, :])
    return out


# revision 61
# speedup vs baseline: 1.1777x; 1.0055x over previous
"""Multi-head self-attention Trainium2 kernel (8 NeuronCores).

Sharding: 8 cores = 4 batches x 2 head-halves (6 heads each). Per core:
QKV projections run as fp8(e4m3) hi+lo DoubleRow matmuls (host splits X
and W into hi/lo fp8 pairs; the lo*lo term is dropped), scores S^T = K.Q^T
in fp16 per (head, key-tile, q-block), P = exp(S/8) on ScalarE straight
out of 3-bank PSUM groups, ctx in the natural [q, d] orientation
(lhsT = P^T slab slices, rhs = V with a ones column for the softmax
denominator) accumulated per (head, q-block) in a time-split PSUM bank,
normalization + PE transpose into ctx^T, and the out-projection
Y_partial = ctx @ Wo[:, cols]^T in fp16. The host sums the two per-batch
partials and adds the output bias.

Scheduling: each unit's ctx matmuls trail one unit behind the score/exp
stream (head-major combo order so the shared ctx PSUM bank is time-split
between the two heads); projections for the next head pair, V tiles,
transposes and out-projection slices drip into the rotation stream.
"""

import numpy as np

B, S, E, H, D = 4, 2048, 768, 12, 64
NCORES = 8

_cache = {}


def _emit(nc, tc, ctx, io):
    import concourse.mybir as mybir

    fp32 = mybir.dt.float32
    fp16 = mybir.dt.float16
    fp8 = mybir.dt.float8e4
    i16 = mybir.dt.int16
    Exp = mybir.ActivationFunctionType.Exp
    DR = mybir.MatmulPerfMode.DoubleRow

    Scfg, Ecfg, EL = 2048, 768, 384
    NKT = Ecfg // 128           # contraction tiles over embed dim (6)
    NS = Scfg // 128            # key tiles (16)
    NQB = Scfg // 512           # q-blocks (4)
    HP = EL // 128              # head pairs (3)
    ROT = 3                     # exp group size in PSUM banks
    NCOMBO = 2 * NS             # (head, k-tile) combos per unit (32)
    NROT = (NCOMBO + ROT - 1) // ROT  # 11
    # Rotations offloaded to a DVE exp approximation: two phase-shifted
    # int16-bitcast Schraudolph evaluations (B-phase magnitude-compensated
    # by a 1/sqrt(2)-scaled V copy) average to ~0.8% max element error.
    SCH = frozenset()
    SCH_A = float(1024.0 * np.log2(np.e) / 8.0)
    SCH_BA = float(1024.0 * (15.0 - 0.054) + 0.5 - 1024.0)
    SCH_BB = SCH_BA + 512.0

    X8, W8Q, W8QS, W8K, W8KS, W8V, W8VS, WOT, BQ, BK, BV, IDT, Y = (
        io["X8"], io["W8Q"], io["W8QS"], io["W8K"], io["W8KS"], io["W8V"],
        io["W8VS"], io["WOT"], io["BQ"], io["BK"], io["BV"], io["IDT"],
        io["Y"],
    )

    consts = ctx.enter_context(tc.tile_pool(name="consts", bufs=1))
    wpool = ctx.enter_context(tc.tile_pool(name="wpool", bufs=1))
    xpool = ctx.enter_context(tc.tile_pool(name="xpool", bufs=1))
    qkpool = ctx.enter_context(tc.tile_pool(name="qkpool", bufs=4))
    vpool = ctx.enter_context(tc.tile_pool(name="vpool", bufs=1))
    spool = ctx.enter_context(tc.tile_pool(name="spool", bufs=27))
    cpool = ctx.enter_context(tc.tile_pool(name="cpool", bufs=1))
    npool = ctx.enter_context(tc.tile_pool(name="npool", bufs=4))
    ypool = ctx.enter_context(tc.tile_pool(name="ypool", bufs=2))
    psum_g = ctx.enter_context(tc.tile_pool(name="psum_g", bufs=2, space="PSUM"))
    psum_c = ctx.enter_context(tc.tile_pool(name="psum_c", bufs=1, space="PSUM"))
    psum_m = ctx.enter_context(tc.tile_pool(name="psum_m", bufs=1, space="PSUM"))

    # ---- weights / constants to SBUF (critical-path tensors first) ----
    # DMA issue order = first-projection critical path: weights+chunk0,
    # then the (tiny) bias the first drain needs, then the cross weights
    # DMA order = the first projection's critical chain: everything q0
    # needs (w8q, chunk 0, cross weights, bias) streams before w8k so the
    # PE chews on Q while K's weights arrive
    w8q_sb = wpool.tile([128, NKT * 2 * EL], fp8, name="w8q_sb")
    nc.sync.dma_start(out=w8q_sb, in_=W8Q)
    x8_sb = xpool.tile([128, NKT * 2 * Scfg], fp8, name="x8_sb")
    x4s = x8_sb.rearrange("p (k h t) -> p k h t", k=NKT, h=2)
    x4d = X8.rearrange("p (k h t) -> p k h t", k=NKT, h=2)

    def xchunk(tc_):
        tsl = slice(tc_ * 512, (tc_ + 1) * 512)
        nc.sync.dma_start(out=x4s[:, :, :, tsl], in_=x4d[:, :, :, tsl])

    xchunk(0)
    w8qs_sb = wpool.tile([128, NKT * 2 * EL], fp8, name="w8qs_sb")
    nc.sync.dma_start(out=w8qs_sb, in_=W8QS)
    bq_sb = consts.tile([128, HP], fp32, name="bq_sb")
    nc.sync.dma_start(out=bq_sb, in_=BQ)
    bk_sb = consts.tile([128, HP], fp32, name="bk_sb")
    nc.sync.dma_start(out=bk_sb, in_=BK)
    w8k_sb = wpool.tile([128, NKT * 2 * EL], fp8, name="w8k_sb")
    nc.sync.dma_start(out=w8k_sb, in_=W8K)
    w8ks_sb = wpool.tile([128, NKT * 2 * EL], fp8, name="w8ks_sb")
    nc.sync.dma_start(out=w8ks_sb, in_=W8KS)
    for tc_ in range(1, 4):
        xchunk(tc_)
    bv_sb = consts.tile([128, EL], fp32, name="bv_sb")
    nc.sync.dma_start(out=bv_sb, in_=BV)
    idt_sb = consts.tile([128, 128], fp16, name="idt_sb")
    nc.sync.dma_start(out=idt_sb, in_=IDT)
    w8v_sb = wpool.tile([128, NKT * 2 * EL], fp8, name="w8v_sb")
    nc.sync.dma_start(out=w8v_sb, in_=W8V)
    w8vs_sb = wpool.tile([128, NKT * 2 * EL], fp8, name="w8vs_sb")
    nc.sync.dma_start(out=w8vs_sb, in_=W8VS)

    wo_sb = []
    for hp in range(HP):
        t = wpool.tile([128, Ecfg], fp16, name=f"wo{hp}_sb", tag=f"wo{hp}")
        nc.sync.dma_start(out=t, in_=WOT[hp * 128:(hp + 1) * 128, :])
        wo_sb.append(t)

    # 4D views: [p, ktile, hi/lo, cols]
    x4 = x8_sb.rearrange("p (k h t) -> p k h t", k=NKT, h=2)
    w4q = w8q_sb.rearrange("p (k h c) -> p k h c", k=NKT, h=2)
    w4qs = w8qs_sb.rearrange("p (k h c) -> p k h c", k=NKT, h=2)
    w4k = w8k_sb.rearrange("p (k h c) -> p k h c", k=NKT, h=2)
    w4ks = w8ks_sb.rearrange("p (k h c) -> p k h c", k=NKT, h=2)
    w4v = w8v_sb.rearrange("p (k h c) -> p k h c", k=NKT, h=2)
    w4vs = w8vs_sb.rearrange("p (k h c) -> p k h c", k=NKT, h=2)

    ctxT = []
    for hp in range(HP):
        t = cpool.tile([128, Scfg], fp16, name=f"ctxT{hp}", tag=f"ctx{hp}")
        ctxT.append(t)

    import collections
    pending = collections.deque()
    chain = [None]   # a closure that MUST be the next pop (its PSUM bank is
                     # mid-accumulation — nothing may allocate in between)

    def drip(n=1):
        for _ in range(n):
            if chain[0] is not None:
                cl, chain[0] = chain[0], None
            elif pending:
                cl = pending.popleft()
            else:
                return
            cl()

    # ---- fp8 hi/lo DoubleRow projection closures ----
    def build_proj(hp):
        qt = qkpool.tile([128, Scfg], fp16, name=f"qt{hp}", tag="qt")
        kt = qkpool.tile([128, Scfg], fp16, name=f"kt{hp}", tag="kt")
        hsl = slice(hp * 128, (hp + 1) * 128)
        closures = []
        for dst, w4, w4s, bsb in ((qt, w4q, w4qs, bq_sb), (kt, w4k, w4ks, bk_sb)):
            for nb in range(NQB):
                def cl(dst=dst, w4=w4, w4s=w4s, bsb=bsb, nb=nb, hp=hp, hsl=hsl):
                    tb = slice(nb * 512, (nb + 1) * 512)
                    ps = psum_m.tile([128, 512], fp32, name=f"pj{hp}_{nb}",
                                     tag="m")
                    for j, k in enumerate(range(0, NKT, 2)):
                        nc.tensor.matmul(
                            ps, lhsT=w4[:, k:k + 2, 0, hsl],
                            rhs=x4[:, k:k + 2, 0, tb],
                            start=(j == 0), stop=False, perf_mode=DR)

                    def cl2(ps=ps):
                        for k in range(NKT):
                            nc.tensor.matmul(
                                ps, lhsT=w4s[:, k, :, hsl],
                                rhs=x4[:, k, :, tb],
                                start=False, stop=(k == NKT - 1), perf_mode=DR)
                        nc.vector.tensor_scalar(
                            dst[:, tb], ps, 1.0 / 64.0, bsb[:, hp:hp + 1],
                            mybir.AluOpType.mult, mybir.AluOpType.add)
                    chain[0] = cl2
                closures.append(cl)
        return qt, kt, closures

    # ---- V for ALL heads: per key-tile [128 tok, 6*65] fp16, ones col per
    # head for the softmax denominator; v2 = V/sqrt(2) feeds the B-phase of
    # the DVE exp approximation ----
    v_sb = [vpool.tile([128, 65 * 2 * HP], fp16, name=f"v_{si}", tag=f"v{si}")
            for si in range(NS)]
    sch_k = sorted({ci % NS for r in SCH
                    for ci in range(ROT * r, min(ROT * r + ROT, NCOMBO))})
    v2_sb = {si: vpool.tile([128, 65 * 2 * HP], fp16, name=f"v2_{si}",
                            tag=f"v2{si}")
             for si in sch_k}

    v_done = [False] * NS

    def build_v():
        closures = []
        for si in range(NS):
            def cl(si=si, vt=v_sb[si]):
                ssl = slice(si * 128, (si + 1) * 128)
                nc.vector.memset(vt, 1.0)
                ps = psum_m.tile([128, EL], fp32, name=f"pv{si}", tag="m")
                for j, k in enumerate(range(0, NKT, 2)):
                    nc.tensor.matmul(
                        ps, lhsT=x4[:, k:k + 2, 0, ssl],
                        rhs=w4v[:, k:k + 2, 0, :],
                        start=(j == 0), stop=False, perf_mode=DR)

                def cl2(si=si, vt=vt, ps=ps):
                    for k in range(NKT):
                        nc.tensor.matmul(
                            ps, lhsT=x4[:, k, :, ssl],
                            rhs=w4vs[:, k, :, :],
                            start=False, stop=(k == NKT - 1), perf_mode=DR)
                    nc.vector.scalar_tensor_tensor(
                        vt.rearrange("p (h w) -> p h w", w=65)[:, :, 0:64],
                        ps.rearrange("p (h w) -> p h w", w=64),
                        1.0 / 64.0,
                        bv_sb.rearrange("p (h w) -> p h w", w=64),
                        mybir.AluOpType.mult, mybir.AluOpType.add)
                    if si in v2_sb:
                        nc.vector.tensor_scalar_mul(v2_sb[si], vt,
                                                    float(2.0 ** -0.5))
                    v_done[si] = True
                chain[0] = cl2
            closures.append(cl)
        return closures

    # ---- out-projection: Y[si] = ctx @ Wo_loc^T (fp16), fp16 DMA out ----
    def build_y(si, gp=False):
        # two closures (one per 384-col chunk) so the drip stays fine-grained
        cell = {}

        def chunk(nh, si=si):
            if nh == 0:
                cell["y"] = ypool.tile([128, Ecfg], fp16, name=f"y{si}",
                                       tag="y")
            y_sb = cell["y"]
            # tail closures alternate into the ctx bank (free once the
            # final normalize has drained) to overlap drains with matmuls
            p = psum_c if (gp and nh == 1) else psum_m
            yps = p.tile([128, 384], fp32, name=f"yp{si}_{nh}",
                         tag="c" if p is psum_c else "m")
            for hp in range(HP):
                nc.tensor.matmul(
                    yps, lhsT=ctxT[hp][:, si * 128:(si + 1) * 128],
                    rhs=wo_sb[hp][:, nh * 384:(nh + 1) * 384],
                    start=(hp == 0), stop=(hp == HP - 1))
            if gp and nh == 0:
                # tail: ACT is idle — split the drains across engines
                nc.scalar.copy(y_sb[:, nh * 384:(nh + 1) * 384], yps)
            else:
                nc.vector.tensor_copy(y_sb[:, nh * 384:(nh + 1) * 384], yps)
            if nh == 1:
                nc.sync.dma_start(out=Y[si * 128:(si + 1) * 128, :], in_=y_sb)
        return [lambda nh=nh: chunk(nh) for nh in range(2)]

    y_by_qb = {qb: [cl for si in range(qb * (NS // NQB),
                                       (qb + 1) * (NS // NQB))
                    for cl in build_y(si, gp=(qb == NQB - 1))]
               for qb in range(NQB)}

    # ---- trailing ctx: natural [q, 65] orientation, accumulated per
    # (head, q-block) in a time-split PSUM bank; on the head's last k-tile,
    # normalize (DVE) and queue the PE transpose into the drip stream so
    # the PE never waits on the normalize chain ----
    def build_transp(hp_u, qb_u, hh, ctxn):
        def cl():
            tp = psum_m.tile([64, 512], fp16, name=f"tp{hp_u}_{qb_u}_{hh}",
                             tag="m")
            for qt in range(4):
                nc.tensor.matmul(
                    tp[:, qt * 128:qt * 128 + 128],
                    lhsT=ctxn[:, qt * 64:qt * 64 + 64],
                    rhs=idt_sb, is_transpose=True)
            nc.vector.tensor_copy(
                ctxT[hp_u][hh * 64:hh * 64 + 64,
                           qb_u * 512:qb_u * 512 + 512], tp)
        return cl

    # PSUM start=True lazily zeroes the whole 2KB bank, so the four 65-col
    # ctx regions sharing a bank must accumulate strictly one-after-another
    # (qt-major): a region's 16-step accumulation may not interleave with a
    # sibling region's start.
    def ctx_item(u, hh, qt, k):
        h6 = 2 * u["hp"] + hh
        if qt == 0 and k == 0:
            u["cps"] = psum_c.tile([128, 4 * 65], fp32,
                                   name=f"cps{u['hp']}_{u['qb']}_{hh}",
                                   tag="c")
        cps = u["cps"]
        r, j = divmod(hh * NS + k, ROT)
        slab = u["slabs"][r]
        csl = slice(qt * 65, qt * 65 + 65)
        ssl = slice(j * 512 + qt * 128, j * 512 + qt * 128 + 128)
        if isinstance(slab, tuple):
            sa, sb = slab
            nc.tensor.matmul(cps[:, csl], lhsT=sa[:, ssl],
                             rhs=v_sb[k][:, h6 * 65:h6 * 65 + 65],
                             start=(k == 0), stop=False)
            nc.tensor.matmul(cps[:, csl], lhsT=sb[:, ssl],
                             rhs=v2_sb[k][:, h6 * 65:h6 * 65 + 65],
                             start=False, stop=(k == NS - 1))
        else:
            nc.tensor.matmul(
                cps[:, csl], lhsT=slab[:, ssl],
                rhs=v_sb[k][:, h6 * 65:h6 * 65 + 65],
                start=(k == 0), stop=(k == NS - 1))
        if qt == 3 and k == NS - 1:
            hp_u, qb_u = u["hp"], u["qb"]
            c3 = cps.rearrange("p (qt c) -> p qt c", c=65)
            rc = npool.tile([128, 4], fp32, name=f"rc{hp_u}_{qb_u}_{hh}",
                            tag="rc")
            nc.vector.reciprocal(rc, c3[:, :, 64:65])
            ctxn = npool.tile([128, 256], fp16, name=f"cn{hp_u}_{qb_u}_{hh}",
                              tag="cn")
            for q2 in range(4):
                nc.vector.tensor_scalar_mul(
                    ctxn[:, q2 * 64:q2 * 64 + 64], c3[:, q2, 0:64],
                    rc[:, q2:q2 + 1])
            pending.appendleft(build_transp(hp_u, qb_u, hh, ctxn))

    # global rotation history for the lag-3 ctx trail; trail items are
    # single matmuls (unit, hh, qt, k) gated on their slab's exp rotation
    LAG = 3
    hist = []            # cumulative combos available after each global rot
    trail_q = collections.deque()   # (unit, hh, qt, k, gate)
    slow = collections.deque()      # heavy closures, paced 1 per 3 rotations

    def trail_to(target, cap=24):
        # pause before a fresh PSUM bank (hh, qt=0, k=0) so the previous
        # half-unit's normalize has a rotation of shadow
        emitted = 0
        while trail_q and emitted < cap:
            u, hh, qt, k, gate = trail_q[0]
            if gate >= target or not v_done[k]:
                break
            if emitted and qt == 0 and k == 0:
                break
            trail_q.popleft()
            ctx_item(u, hh, qt, k)
            emitted += 1

    cur = build_proj(0)
    vcl = build_v()
    qt_dbg = None
    for hp in range(HP):
        qt, kt, closures = cur
        if hp == 0:
            qt_dbg = (qt, kt)
        if hp == 0:
            # minimal prefix so unit (0,0) can start; keys first, V paced
            # to arrive before the (deferred) ctx trail consumes it.
            for cl in (closures[0], closures[NQB]):
                cl()
                if chain[0] is not None:
                    c2, chain[0] = chain[0], None
                    c2()
            krest = closures[NQB + 1:2 * NQB]
            qrest = closures[1:NQB]
            pend0 = (krest[:2] + qrest[:1] + krest[2:] + qrest[1:] + vcl)
            pending.extend(pend0)
        cur = build_proj(hp + 1) if hp + 1 < HP else None
        if cur is not None:
            pending.extend(cur[2])

        if hp == 1 and "DQT" in io:
            nc.sync.dma_start(out=io["DQT"], in_=qt_dbg[0])
            nc.sync.dma_start(out=io["DKT"], in_=qt_dbg[1])
            nc.sync.dma_start(out=io["DV0"], in_=v_sb[0])
        for qb in range(NQB):
            qsl = slice(qb * 512, qb * 512 + 512)
            unit = dict(hp=hp, qb=qb, slabs=[], cps=None)
            first_unit = (hp == 0 and qb == 0)
            base = NCOMBO * (hp * NQB + qb)
            for hh_ in range(2):
                for qt_ in range(4):
                    for k_ in range(NS):
                        gate = base + hh_ * NS + (k_ if qt_ == 0 else NS - 1)
                        trail_q.append((unit, hh_, qt_, k_, gate))
            for r in range(NROT):
                if hp == HP - 1 and qb >= 1 and r == 6:
                    pending.extend(y_by_qb[qb - 1])
                cis = range(ROT * r, min(ROT * r + ROT, NCOMBO))
                n = len(cis)
                g = psum_g.tile([128, 512 * ROT], fp32,
                                name=f"g{hp}_{qb}_{r}", tag="g")
                for j, ci in enumerate(cis):
                    hh, k = divmod(ci, NS)
                    nc.tensor.matmul(
                        g[:, j * 512:(j + 1) * 512],
                        lhsT=kt[hh * 64:(hh + 1) * 64, k * 128:(k + 1) * 128],
                        rhs=qt[hh * 64:(hh + 1) * 64, qsl],
                        start=True, stop=True)
                drip(1)
                slab = spool.tile([128, 512 * ROT], fp16,
                                  name=f"s{hp}_{qb}_{r}", tag="slab")
                if r in SCH:
                    slab_b = spool.tile([128, 512 * ROT], fp16,
                                        name=f"sb{hp}_{qb}_{r}", tag="slab")
                    nc.vector.tensor_scalar(
                        slab[:, :n * 512].bitcast(i16), g[:, :n * 512],
                        SCH_A, SCH_BA, mybir.AluOpType.mult,
                        mybir.AluOpType.add)
                    nc.vector.tensor_scalar(
                        slab_b[:, :n * 512].bitcast(i16), g[:, :n * 512],
                        SCH_A, SCH_BB, mybir.AluOpType.mult,
                        mybir.AluOpType.add)
                    unit["slabs"].append((slab, slab_b))
                else:
                    nc.scalar.activation(slab[:, :n * 512], g[:, :n * 512],
                                         Exp, scale=0.125)
                    unit["slabs"].append(slab)
                hist.append((hist[-1] if hist else 0) + n)
                gi = len(hist) - 1
                if gi >= 2 * LAG:
                    trail_to(hist[gi - LAG])
                drip(2 if first_unit else 0)

    # drain the remaining ctx matmuls, dripping in the leftover closures
    nt = 0
    while trail_q:
        u, hh, qt, k, gate = trail_q.popleft()
        ctx_item(u, hh, qt, k)
        nt += 1
        if nt % 4 == 0:
            drip(1)
    pending.extend(y_by_qb[NQB - 1])
    drip(10000)
    if "DCT" in io:
        for hp in range(HP):
            nc.sync.dma_start(
                out=io["DCT"][hp * 128:(hp + 1) * 128, :], in_=ctxT[hp])


def _build():
    import contextlib
    import concourse.mybir as mybir
    import concourse.tile as tile
    from concourse import bacc

    fp32, fp16 = mybir.dt.float32, mybir.dt.float16
    fp8 = mybir.dt.float8e4
    Scfg, Ecfg, EL, NKT, HP = 2048, 768, 384, 6, 3

    nc = bacc.Bacc("TRN2", target_bir_lowering=False, debug=False,
                   num_devices=NCORES)
    io = {
        "X8": nc.dram_tensor("X8", [128, NKT * 2 * Scfg], fp8,
                             kind="ExternalInput").ap(),
        "W8Q": nc.dram_tensor("W8Q", [128, NKT * 2 * EL], fp8,
                              kind="ExternalInput").ap(),
        "W8QS": nc.dram_tensor("W8QS", [128, NKT * 2 * EL], fp8,
                               kind="ExternalInput").ap(),
        "W8K": nc.dram_tensor("W8K", [128, NKT * 2 * EL], fp8,
                              kind="ExternalInput").ap(),
        "W8KS": nc.dram_tensor("W8KS", [128, NKT * 2 * EL], fp8,
                               kind="ExternalInput").ap(),
        "W8V": nc.dram_tensor("W8V", [128, NKT * 2 * EL], fp8,
                              kind="ExternalInput").ap(),
        "W8VS": nc.dram_tensor("W8VS", [128, NKT * 2 * EL], fp8,
                               kind="ExternalInput").ap(),
        "WOT": nc.dram_tensor("WOT", [EL, Ecfg], fp16,
                              kind="ExternalInput").ap(),
        "BQ": nc.dram_tensor("BQ", [128, HP], fp32, kind="ExternalInput").ap(),
        "BK": nc.dram_tensor("BK", [128, HP], fp32, kind="ExternalInput").ap(),
        "BV": nc.dram_tensor("BV", [128, EL], fp32, kind="ExternalInput").ap(),
        "IDT": nc.dram_tensor("IDT", [128, 128], fp16,
                              kind="ExternalInput").ap(),
        "Y": nc.dram_tensor("Y", [Scfg, Ecfg], fp16,
                            kind="ExternalOutput").ap(),
    }
    with tile.TileContext(nc) as tc:
        with contextlib.ExitStack() as ctx:
            _emit(nc, tc, ctx, io)
    nc.compile()
    return nc


def _get_program():
    if "full" not in _cache:
        _cache["full"] = _build()
    return _cache["full"]


def _hilo_pack(a):
    """[128, k, cols] fp32 -> [128, k, 2, cols] (hi, lo) fp8 pair + swapped."""
    import ml_dtypes
    e4 = ml_dtypes.float8_e4m3
    hi = a.astype(e4)
    lo = (a - hi.astype(np.float32)).astype(e4)
    norm = np.stack([hi, lo], axis=2)
    swap = np.stack([lo, hi], axis=2)
    return norm, swap


def _k_major(a, ncols):
    """[rows=k*128, ncols] -> [128, k, ncols] (contraction-tile-major)."""
    k = a.shape[0] // 128
    return np.ascontiguousarray(a.reshape(k, 128, ncols).transpose(1, 0, 2))


def _half_inputs(half, Wq, bq, Wk, bk, Wv, bv, Wo):
    import ml_dtypes
    f16 = np.float16
    e0 = 384 * half
    ecols = slice(e0, e0 + 384)
    out = {}
    for nm, W in (("Q", Wq), ("K", Wk), ("V", Wv)):
        # x64 pre-scale keeps the lo residual above e4m3's subnormal floor;
        # the PSUM drain multiplies by 1/64.
        wt = _k_major(np.ascontiguousarray(W[ecols, :].T) * 64.0, 384)
        norm, swap = _hilo_pack(wt)
        out[f"W8{nm}"] = norm.reshape(128, -1)
        out[f"W8{nm}S"] = swap.reshape(128, -1)
    out["WOT"] = np.ascontiguousarray(Wo[:, ecols].T).astype(f16)
    out["BQ"] = np.ascontiguousarray(bq[ecols].reshape(3, 128).T).astype(np.float32)
    out["BK"] = np.ascontiguousarray(bk[ecols].reshape(3, 128).T).astype(np.float32)
    out["BV"] = np.ascontiguousarray(
        np.broadcast_to(bv[ecols], (128, 384))).astype(np.float32)
    out["IDT"] = np.eye(128, dtype=f16)
    return out


def kernel(X, Wq, bq, Wk, bk, Wv, bv, Wo, bo):
    from concourse.bass_utils import run_bass_kernel_spmd

    X, Wq, bq, Wk, bk, Wv, bv, Wo, bo = [
        np.asarray(a, dtype=np.float32)
        for a in (X, Wq, bq, Wk, bk, Wv, bv, Wo, bo)
    ]
    nc = _get_program()
    halves = [_half_inputs(h, Wq, bq, Wk, bk, Wv, bv, Wo) for h in range(2)]
    x8s = []
    for b in range(B):
        xt = _k_major(np.ascontiguousarray(X[b].T), 2048)
        norm, _ = _hilo_pack(xt)
        x8s.append(norm.reshape(128, -1))
    in_maps = [dict(halves[c % 2], X8=x8s[c // 2]) for c in range(NCORES)]
    res = run_bass_kernel_spmd(nc, in_maps, list(range(NCORES)))
    out = np.empty((B, S, E), np.float32)
    for b in range(B):
        out[b] = (res.results[2 * b]["Y"].astype(np.float32)
                  + res.results[2 * b + 1]["Y"].astype(np.float32)
                  + bo[None, :])
    return out


# revision 84
# speedup vs baseline: 1.1979x; 1.0172x over previous
"""Multi-head self-attention Trainium2 kernel (8 NeuronCores).

Sharding: 8 cores = 4 batches x 2 head-halves (6 heads each). Per core:
QKV projections run as fp8(e4m3) hi+lo DoubleRow matmuls (host splits X
and W into hi/lo fp8 pairs; the lo*lo term is dropped), scores S^T = K.Q^T
in fp16 per (head, key-tile, q-block), P = exp(S/8) on ScalarE straight
out of 3-bank PSUM groups, ctx in the natural [q, d] orientation
(lhsT = P^T slab slices, rhs = V with a ones column for the softmax
denominator) accumulated per (head, q-block) in a time-split PSUM bank,
normalization + PE transpose into ctx^T, and the out-projection
Y_partial = ctx @ Wo[:, cols]^T in fp16. The host sums the two per-batch
partials and adds the output bias.

Scheduling: each unit's ctx matmuls trail one unit behind the score/exp
stream (head-major combo order so the shared ctx PSUM bank is time-split
between the two heads); projections for the next head pair, V tiles,
transposes and out-projection slices drip into the rotation stream.
"""

import numpy as np

B, S, E, H, D = 4, 2048, 768, 12, 64
NCORES = 8

_cache = {}


def _emit(nc, tc, ctx, io):
    import concourse.mybir as mybir

    fp32 = mybir.dt.float32
    fp16 = mybir.dt.float16
    fp8 = mybir.dt.float8e4
    i16 = mybir.dt.int16
    Exp = mybir.ActivationFunctionType.Exp
    DR = mybir.MatmulPerfMode.DoubleRow

    Scfg, Ecfg, EL = 2048, 768, 384
    NKT = Ecfg // 128           # contraction tiles over embed dim (6)
    NS = Scfg // 128            # key tiles (16)
    NQB = Scfg // 512           # q-blocks (4)
    HP = EL // 128              # head pairs (3)
    ROT = 3                     # exp group size in PSUM banks
    NCOMBO = 2 * NS             # (head, k-tile) combos per unit (32)
    NROT = (NCOMBO + ROT - 1) // ROT  # 11
    # Rotations offloaded off the ScalarE: two phase-shifted int16-bitcast
    # Schraudolph evaluations (DVE, reading PSUM) averaged on the otherwise
    # idle GPSIMD engine ((B * 2^-0.5) + A, SBUF-only) — ~0.8% max element
    # error on the offloaded keys.
    SCH = frozenset()
    SCH_A = float(1024.0 * np.log2(np.e) / 8.0)
    SCH_BA = float(1024.0 * (15.0 - 0.054) + 0.5 - 1024.0)
    SCH_BB = SCH_BA + 512.0

    X8, W8Q, W8QS, W8K, W8KS, W8V, W8VS, WOT, BQ, BK, BV, IDT, Y = (
        io["X8"], io["W8Q"], io["W8QS"], io["W8K"], io["W8KS"], io["W8V"],
        io["W8VS"], io["WOT"], io["BQ"], io["BK"], io["BV"], io["IDT"],
        io["Y"],
    )

    consts = ctx.enter_context(tc.tile_pool(name="consts", bufs=1))
    wpool = ctx.enter_context(tc.tile_pool(name="wpool", bufs=1))
    xpool = ctx.enter_context(tc.tile_pool(name="xpool", bufs=1))
    qkpool = ctx.enter_context(tc.tile_pool(name="qkpool", bufs=4))
    vpool = ctx.enter_context(tc.tile_pool(name="vpool", bufs=1))
    spool = ctx.enter_context(tc.tile_pool(name="spool", bufs=27))
    schpool = ctx.enter_context(tc.tile_pool(name="schpool", bufs=2))
    cpool = ctx.enter_context(tc.tile_pool(name="cpool", bufs=1))
    npool = ctx.enter_context(tc.tile_pool(name="npool", bufs=4))
    ypool = ctx.enter_context(tc.tile_pool(name="ypool", bufs=2))
    psum_g = ctx.enter_context(tc.tile_pool(name="psum_g", bufs=2, space="PSUM"))
    psum_c = ctx.enter_context(tc.tile_pool(name="psum_c", bufs=1, space="PSUM"))
    psum_m = ctx.enter_context(tc.tile_pool(name="psum_m", bufs=1, space="PSUM"))

    # ---- weights / constants to SBUF (critical-path tensors first) ----
    # DMA issue order = first-projection critical path: weights+chunk0,
    # then the (tiny) bias the first drain needs, then the cross weights
    # DMA order = the first projection's critical chain: everything q0
    # needs (w8q, chunk 0, cross weights, bias) streams before w8k so the
    # PE chews on Q while K's weights arrive
    w8q_sb = wpool.tile([128, NKT * 2 * EL], fp8, name="w8q_sb")
    nc.sync.dma_start(out=w8q_sb, in_=W8Q)
    x8_sb = xpool.tile([128, NKT * 2 * Scfg], fp8, name="x8_sb")
    x4s = x8_sb.rearrange("p (k h t) -> p k h t", k=NKT, h=2)
    x4d = X8.rearrange("p (k h t) -> p k h t", k=NKT, h=2)

    def xchunk(tc_):
        tsl = slice(tc_ * 512, (tc_ + 1) * 512)
        nc.sync.dma_start(out=x4s[:, :, :, tsl], in_=x4d[:, :, :, tsl])

    xchunk(0)
    w8qs_sb = wpool.tile([128, NKT * 2 * EL], fp8, name="w8qs_sb")
    nc.sync.dma_start(out=w8qs_sb, in_=W8QS)
    bq_sb = consts.tile([128, HP], fp32, name="bq_sb")
    nc.sync.dma_start(out=bq_sb, in_=BQ)
    bk_sb = consts.tile([128, HP], fp32, name="bk_sb")
    nc.sync.dma_start(out=bk_sb, in_=BK)
    w8k_sb = wpool.tile([128, NKT * 2 * EL], fp8, name="w8k_sb")
    nc.sync.dma_start(out=w8k_sb, in_=W8K)
    w8ks_sb = wpool.tile([128, NKT * 2 * EL], fp8, name="w8ks_sb")
    nc.sync.dma_start(out=w8ks_sb, in_=W8KS)
    for tc_ in range(1, 4):
        xchunk(tc_)
    bv_sb = consts.tile([128, EL], fp32, name="bv_sb")
    nc.sync.dma_start(out=bv_sb, in_=BV)
    idt_sb = consts.tile([128, 128], fp16, name="idt_sb")
    nc.sync.dma_start(out=idt_sb, in_=IDT)
    w8v_sb = wpool.tile([128, NKT * 2 * EL], fp8, name="w8v_sb")
    nc.sync.dma_start(out=w8v_sb, in_=W8V)
    w8vs_sb = wpool.tile([128, NKT * 2 * EL], fp8, name="w8vs_sb")
    nc.sync.dma_start(out=w8vs_sb, in_=W8VS)

    wo_sb = []
    for hp in range(HP):
        t = wpool.tile([128, Ecfg], fp16, name=f"wo{hp}_sb", tag=f"wo{hp}")
        nc.sync.dma_start(out=t, in_=WOT[hp * 128:(hp + 1) * 128, :])
        wo_sb.append(t)

    # 4D views: [p, ktile, hi/lo, cols]
    x4 = x8_sb.rearrange("p (k h t) -> p k h t", k=NKT, h=2)
    w4q = w8q_sb.rearrange("p (k h c) -> p k h c", k=NKT, h=2)
    w4qs = w8qs_sb.rearrange("p (k h c) -> p k h c", k=NKT, h=2)
    w4k = w8k_sb.rearrange("p (k h c) -> p k h c", k=NKT, h=2)
    w4ks = w8ks_sb.rearrange("p (k h c) -> p k h c", k=NKT, h=2)
    w4v = w8v_sb.rearrange("p (k h c) -> p k h c", k=NKT, h=2)
    w4vs = w8vs_sb.rearrange("p (k h c) -> p k h c", k=NKT, h=2)

    ctxT = []
    for hp in range(HP):
        t = cpool.tile([128, Scfg], fp16, name=f"ctxT{hp}", tag=f"ctx{hp}")
        ctxT.append(t)

    import collections
    pending = collections.deque()
    chain = [None]   # a closure that MUST be the next pop (its PSUM bank is
                     # mid-accumulation — nothing may allocate in between)

    def drip(n=1):
        for _ in range(n):
            if chain[0] is not None:
                cl, chain[0] = chain[0], None
            elif pending:
                cl = pending.popleft()
            else:
                return
            cl()

    # ---- fp8 hi/lo DoubleRow projection closures ----
    def build_proj(hp):
        qt = qkpool.tile([128, Scfg], fp16, name=f"qt{hp}", tag="qt")
        kt = qkpool.tile([128, Scfg], fp16, name=f"kt{hp}", tag="kt")
        hsl = slice(hp * 128, (hp + 1) * 128)
        closures = []
        for dst, w4, w4s, bsb in ((qt, w4q, w4qs, bq_sb), (kt, w4k, w4ks, bk_sb)):
            for nb in range(NQB):
                def cl(dst=dst, w4=w4, w4s=w4s, bsb=bsb, nb=nb, hp=hp, hsl=hsl):
                    tb = slice(nb * 512, (nb + 1) * 512)
                    ps = psum_m.tile([128, 512], fp32, name=f"pj{hp}_{nb}",
                                     tag="m")
                    for j, k in enumerate(range(0, NKT, 2)):
                        nc.tensor.matmul(
                            ps, lhsT=w4[:, k:k + 2, 0, hsl],
                            rhs=x4[:, k:k + 2, 0, tb],
                            start=(j == 0), stop=False, perf_mode=DR)

                    def cl2(ps=ps):
                        for k in range(NKT):
                            nc.tensor.matmul(
                                ps, lhsT=w4s[:, k, :, hsl],
                                rhs=x4[:, k, :, tb],
                                start=False, stop=(k == NKT - 1), perf_mode=DR)
                        nc.vector.tensor_scalar(
                            dst[:, tb], ps, 1.0 / 64.0, bsb[:, hp:hp + 1],
                            mybir.AluOpType.mult, mybir.AluOpType.add)
                    chain[0] = cl2
                closures.append(cl)
        return qt, kt, closures

    # ---- V for ALL heads: per key-tile [128 tok, 6*65] fp16, ones col per
    # head for the softmax denominator; v2 = V/sqrt(2) feeds the B-phase of
    # the DVE exp approximation ----
    v_sb = [vpool.tile([128, 65 * 2 * HP], fp16, name=f"v_{si}", tag=f"v{si}")
            for si in range(NS)]
    sch_k = sorted({ci % NS for r in SCH
                    for ci in range(ROT * r, min(ROT * r + ROT, NCOMBO))})
    v2_sb = {si: vpool.tile([128, 65 * 2 * HP], fp16, name=f"v2_{si}",
                            tag=f"v2{si}")
             for si in sch_k}

    v_done = [False] * NS

    def build_v():
        closures = []
        for si in range(NS):
            def cl(si=si, vt=v_sb[si]):
                ssl = slice(si * 128, (si + 1) * 128)
                nc.vector.memset(vt, 1.0)
                ps = psum_m.tile([128, EL], fp32, name=f"pv{si}", tag="m")
                for j, k in enumerate(range(0, NKT, 2)):
                    nc.tensor.matmul(
                        ps, lhsT=x4[:, k:k + 2, 0, ssl],
                        rhs=w4v[:, k:k + 2, 0, :],
                        start=(j == 0), stop=False, perf_mode=DR)

                def cl2(si=si, vt=vt, ps=ps):
                    for k in range(NKT):
                        nc.tensor.matmul(
                            ps, lhsT=x4[:, k, :, ssl],
                            rhs=w4vs[:, k, :, :],
                            start=False, stop=(k == NKT - 1), perf_mode=DR)
                    nc.vector.scalar_tensor_tensor(
                        vt.rearrange("p (h w) -> p h w", w=65)[:, :, 0:64],
                        ps.rearrange("p (h w) -> p h w", w=64),
                        1.0 / 64.0,
                        bv_sb.rearrange("p (h w) -> p h w", w=64),
                        mybir.AluOpType.mult, mybir.AluOpType.add)
                    if si in v2_sb:
                        nc.vector.tensor_scalar_mul(v2_sb[si], vt,
                                                    float(2.0 ** -0.5))
                    v_done[si] = True
                chain[0] = cl2
            closures.append(cl)
        return closures

    # ---- out-projection: Y[si] = ctx @ Wo_loc^T (fp16), fp16 DMA out ----
    def build_y(si, gp=False):
        # two closures (one per 384-col chunk) so the drip stays fine-grained
        cell = {}

        def chunk(nh, si=si):
            if nh == 0:
                cell["y"] = ypool.tile([128, Ecfg], fp16, name=f"y{si}",
                                       tag="y")
            y_sb = cell["y"]
            # tail closures alternate into the ctx bank (free once the
            # final normalize has drained) to overlap drains with matmuls
            p = psum_c if (gp and nh == 1) else psum_m
            yps = p.tile([128, 384], fp32, name=f"yp{si}_{nh}",
                         tag="c" if p is psum_c else "m")
            for hp in range(HP):
                nc.tensor.matmul(
                    yps, lhsT=ctxT[hp][:, si * 128:(si + 1) * 128],
                    rhs=wo_sb[hp][:, nh * 384:(nh + 1) * 384],
                    start=(hp == 0), stop=(hp == HP - 1))
            if gp and nh == 0:
                # tail: ACT is idle — split the drains across engines
                nc.scalar.copy(y_sb[:, nh * 384:(nh + 1) * 384], yps)
            else:
                nc.vector.tensor_copy(y_sb[:, nh * 384:(nh + 1) * 384], yps)
            if nh == 1:
                nc.sync.dma_start(out=Y[si * 128:(si + 1) * 128, :], in_=y_sb)
        return [lambda nh=nh: chunk(nh) for nh in range(2)]

    y_by_qb = {qb: [cl for si in range(qb * (NS // NQB),
                                       (qb + 1) * (NS // NQB))
                    for cl in build_y(si, gp=(qb == NQB - 1))]
               for qb in range(NQB)}

    # ---- trailing ctx: natural [q, 65] orientation, accumulated per
    # (head, q-block) in a time-split PSUM bank; on the head's last k-tile,
    # normalize (DVE) and queue the PE transpose into the drip stream so
    # the PE never waits on the normalize chain ----
    def build_transp(hp_u, qb_u, hh, ctxn):
        def cl():
            tp = psum_m.tile([64, 512], fp16, name=f"tp{hp_u}_{qb_u}_{hh}",
                             tag="m")
            for qt in range(4):
                nc.tensor.matmul(
                    tp[:, qt * 128:qt * 128 + 128],
                    lhsT=ctxn[:, qt * 64:qt * 64 + 64],
                    rhs=idt_sb, is_transpose=True)
            nc.vector.tensor_copy(
                ctxT[hp_u][hh * 64:hh * 64 + 64,
                           qb_u * 512:qb_u * 512 + 512], tp)
            if hp_u == HP - 1 and hh == 1:
                # ctxT for this q-block is complete on every head pair —
                # release its out-projection closures (ordering-safe for any
                # trail lag)
                pending.extend(y_by_qb[qb_u])
        return cl

    # PSUM start=True lazily zeroes the whole 2KB bank, so the four 65-col
    # ctx regions sharing a bank must accumulate strictly one-after-another
    # (qt-major): a region's 16-step accumulation may not interleave with a
    # sibling region's start.
    def ctx_item(u, hh, qt, k):
        h6 = 2 * u["hp"] + hh
        if qt == 0 and k == 0:
            u["cps"] = psum_c.tile([128, 4 * 65], fp32,
                                   name=f"cps{u['hp']}_{u['qb']}_{hh}",
                                   tag="c")
        cps = u["cps"]
        r, j = divmod(hh * NS + k, ROT)
        slab = u["slabs"][r]
        csl = slice(qt * 65, qt * 65 + 65)
        ssl = slice(j * 512 + qt * 128, j * 512 + qt * 128 + 128)
        if isinstance(slab, tuple):
            sa, sb = slab
            nc.tensor.matmul(cps[:, csl], lhsT=sa[:, ssl],
                             rhs=v_sb[k][:, h6 * 65:h6 * 65 + 65],
                             start=(k == 0), stop=False)
            nc.tensor.matmul(cps[:, csl], lhsT=sb[:, ssl],
                             rhs=v2_sb[k][:, h6 * 65:h6 * 65 + 65],
                             start=False, stop=(k == NS - 1))
        else:
            nc.tensor.matmul(
                cps[:, csl], lhsT=slab[:, ssl],
                rhs=v_sb[k][:, h6 * 65:h6 * 65 + 65],
                start=(k == 0), stop=(k == NS - 1))
        if qt == 3 and k == NS - 1:
            hp_u, qb_u = u["hp"], u["qb"]
            c3 = cps.rearrange("p (qt c) -> p qt c", c=65)
            rc = npool.tile([128, 4], fp32, name=f"rc{hp_u}_{qb_u}_{hh}",
                            tag="rc")
            nc.vector.reciprocal(rc, c3[:, :, 64:65])
            ctxn = npool.tile([128, 256], fp16, name=f"cn{hp_u}_{qb_u}_{hh}",
                              tag="cn")
            for q2 in range(4):
                nc.vector.tensor_scalar_mul(
                    ctxn[:, q2 * 64:q2 * 64 + 64], c3[:, q2, 0:64],
                    rc[:, q2:q2 + 1])
            pending.appendleft(build_transp(hp_u, qb_u, hh, ctxn))

    # global rotation history for the lag-3 ctx trail; trail items are
    # single matmuls (unit, hh, qt, k) gated on their slab's exp rotation
    LAG = 12
    hist = []            # cumulative combos available after each global rot
    trail_q = collections.deque()   # (unit, hh, qt, k, gate)
    slow = collections.deque()      # heavy closures, paced 1 per 3 rotations

    def trail_to(target, cap=32):
        # pause before a fresh PSUM bank (hh, qt=0, k=0) so the previous
        # half-unit's normalize has a rotation of shadow
        emitted = 0
        while trail_q and emitted < cap:
            u, hh, qt, k, gate = trail_q[0]
            if gate >= target or not v_done[k]:
                break
            if emitted and qt == 0 and k == 0:
                break
            trail_q.popleft()
            ctx_item(u, hh, qt, k)
            emitted += 1

    cur = build_proj(0)
    vcl = build_v()
    qt_dbg = None
    for hp in range(HP):
        qt, kt, closures = cur
        if hp == 0:
            qt_dbg = (qt, kt)
        if hp == 0:
            # minimal prefix so unit (0,0) can start; keys first, V paced
            # to arrive before the (deferred) ctx trail consumes it.
            for cl in (closures[0], closures[NQB]):
                cl()
                if chain[0] is not None:
                    c2, chain[0] = chain[0], None
                    c2()
            krest = closures[NQB + 1:2 * NQB]
            qrest = closures[1:NQB]
            pend0 = (krest[:2] + qrest[:1] + krest[2:] + qrest[1:] + vcl)
            pending.extend(pend0)
        cur = build_proj(hp + 1) if hp + 1 < HP else None
        if cur is not None:
            pending.extend(cur[2])

        if hp == 1 and "DQT" in io:
            nc.sync.dma_start(out=io["DQT"], in_=qt_dbg[0])
            nc.sync.dma_start(out=io["DKT"], in_=qt_dbg[1])
            nc.sync.dma_start(out=io["DV0"], in_=v_sb[0])
        for qb in range(NQB):
            qsl = slice(qb * 512, qb * 512 + 512)
            unit = dict(hp=hp, qb=qb, slabs=[], cps=None)
            first_unit = (hp == 0 and qb == 0)
            base = NCOMBO * (hp * NQB + qb)
            for hh_ in range(2):
                for qt_ in range(4):
                    for k_ in range(NS):
                        gate = base + hh_ * NS + (k_ if qt_ == 0 else NS - 1)
                        trail_q.append((unit, hh_, qt_, k_, gate))
            for r in range(NROT):
                cis = range(ROT * r, min(ROT * r + ROT, NCOMBO))
                n = len(cis)
                g = psum_g.tile([128, 512 * ROT], fp32,
                                name=f"g{hp}_{qb}_{r}", tag="g")
                for j, ci in enumerate(cis):
                    hh, k = divmod(ci, NS)
                    nc.tensor.matmul(
                        g[:, j * 512:(j + 1) * 512],
                        lhsT=kt[hh * 64:(hh + 1) * 64, k * 128:(k + 1) * 128],
                        rhs=qt[hh * 64:(hh + 1) * 64, qsl],
                        start=True, stop=True)
                drip(1)
                slab = spool.tile([128, 512 * ROT], fp16,
                                  name=f"s{hp}_{qb}_{r}", tag="slab")
                if r in SCH:
                    slab_b = schpool.tile([128, 512 * ROT], fp16,
                                          name=f"sb{hp}_{qb}_{r}", tag="sch")
                    nc.vector.tensor_scalar(
                        slab[:, :n * 512].bitcast(i16), g[:, :n * 512],
                        SCH_A, SCH_BA, mybir.AluOpType.mult,
                        mybir.AluOpType.add)
                    nc.vector.tensor_scalar(
                        slab_b[:, :n * 512].bitcast(i16), g[:, :n * 512],
                        SCH_A, SCH_BB, mybir.AluOpType.mult,
                        mybir.AluOpType.add)
                    unit["slabs"].append((slab, slab_b))
                else:
                    nc.scalar.activation(slab[:, :n * 512], g[:, :n * 512],
                                         Exp, scale=0.125)
                    unit["slabs"].append(slab)
                hist.append((hist[-1] if hist else 0) + n)
                gi = len(hist) - 1
                if gi >= 2 * LAG:
                    trail_to(hist[gi - LAG])
                drip(2 if first_unit else 0)

    # drain the remaining ctx matmuls, dripping in the leftover closures
    nt = 0
    while trail_q:
        u, hh, qt, k, gate = trail_q.popleft()
        ctx_item(u, hh, qt, k)
        nt += 1
        if nt % 4 == 0:
            drip(1)
    drip(10000)
    if "DCT" in io:
        for hp in range(HP):
            nc.sync.dma_start(
                out=io["DCT"][hp * 128:(hp + 1) * 128, :], in_=ctxT[hp])


def _build():
    import contextlib
    import concourse.mybir as mybir
    import concourse.tile as tile
    from concourse import bacc

    fp32, fp16 = mybir.dt.float32, mybir.dt.float16
    fp8 = mybir.dt.float8e4
    Scfg, Ecfg, EL, NKT, HP = 2048, 768, 384, 6, 3

    nc = bacc.Bacc("TRN2", target_bir_lowering=False, debug=False,
                   num_devices=NCORES)
    io = {
        "X8": nc.dram_tensor("X8", [128, NKT * 2 * Scfg], fp8,
                             kind="ExternalInput").ap(),
        "W8Q": nc.dram_tensor("W8Q", [128, NKT * 2 * EL], fp8,
                              kind="ExternalInput").ap(),
        "W8QS": nc.dram_tensor("W8QS", [128, NKT * 2 * EL], fp8,
                               kind="ExternalInput").ap(),
        "W8K": nc.dram_tensor("W8K", [128, NKT * 2 * EL], fp8,
                              kind="ExternalInput").ap(),
        "W8KS": nc.dram_tensor("W8KS", [128, NKT * 2 * EL], fp8,
                               kind="ExternalInput").ap(),
        "W8V": nc.dram_tensor("W8V", [128, NKT * 2 * EL], fp8,
                              kind="ExternalInput").ap(),
        "W8VS": nc.dram_tensor("W8VS", [128, NKT * 2 * EL], fp8,
                               kind="ExternalInput").ap(),
        "WOT": nc.dram_tensor("WOT", [EL, Ecfg], fp16,
                              kind="ExternalInput").ap(),
        "BQ": nc.dram_tensor("BQ", [128, HP], fp32, kind="ExternalInput").ap(),
        "BK": nc.dram_tensor("BK", [128, HP], fp32, kind="ExternalInput").ap(),
        "BV": nc.dram_tensor("BV", [128, EL], fp32, kind="ExternalInput").ap(),
        "IDT": nc.dram_tensor("IDT", [128, 128], fp16,
                              kind="ExternalInput").ap(),
        "Y": nc.dram_tensor("Y", [Scfg, Ecfg], fp16,
                            kind="ExternalOutput").ap(),
    }
    with tile.TileContext(nc) as tc:
        with contextlib.ExitStack() as ctx:
            _emit(nc, tc, ctx, io)
    nc.compile()
    return nc


def _get_program():
    if "full" not in _cache:
        _cache["full"] = _build()
    return _cache["full"]


def _hilo_pack(a):
    """[128, k, cols] fp32 -> [128, k, 2, cols] (hi, lo) fp8 pair + swapped."""
    import ml_dtypes
    e4 = ml_dtypes.float8_e4m3
    hi = a.astype(e4)
    lo = (a - hi.astype(np.float32)).astype(e4)
    norm = np.stack([hi, lo], axis=2)
    swap = np.stack([lo, hi], axis=2)
    return norm, swap


def _k_major(a, ncols):
    """[rows=k*128, ncols] -> [128, k, ncols] (contraction-tile-major)."""
    k = a.shape[0] // 128
    return np.ascontiguousarray(a.reshape(k, 128, ncols).transpose(1, 0, 2))


def _half_inputs(half, Wq, bq, Wk, bk, Wv, bv, Wo):
    import ml_dtypes
    f16 = np.float16
    e0 = 384 * half
    ecols = slice(e0, e0 + 384)
    out = {}
    for nm, W in (("Q", Wq), ("K", Wk), ("V", Wv)):
        # x64 pre-scale keeps the lo residual above e4m3's subnormal floor;
        # the PSUM drain multiplies by 1/64.
        wt = _k_major(np.ascontiguousarray(W[ecols, :].T) * 64.0, 384)
        norm, swap = _hilo_pack(wt)
        out[f"W8{nm}"] = norm.reshape(128, -1)
        out[f"W8{nm}S"] = swap.reshape(128, -1)
    out["WOT"] = np.ascontiguousarray(Wo[:, ecols].T).astype(f16)
    out["BQ"] = np.ascontiguousarray(bq[ecols].reshape(3, 128).T).astype(np.float32)
    out["BK"] = np.ascontiguousarray(bk[ecols].reshape(3, 128).T).astype(np.float32)
    out["BV"] = np.ascontiguousarray(
        np.broadcast_to(bv[ecols], (128, 384))).astype(np.float32)
    out["IDT"] = np.eye(128, dtype=f16)
    return out


def kernel(X, Wq, bq, Wk, bk, Wv, bv, Wo, bo):
    from concourse.bass_utils import run_bass_kernel_spmd

    X, Wq, bq, Wk, bk, Wv, bv, Wo, bo = [
        np.asarray(a, dtype=np.float32)
        for a in (X, Wq, bq, Wk, bk, Wv, bv, Wo, bo)
    ]
    nc = _get_program()
    halves = [_half_inputs(h, Wq, bq, Wk, bk, Wv, bv, Wo) for h in range(2)]
    x8s = []
    for b in range(B):
        xt = _k_major(np.ascontiguousarray(X[b].T), 2048)
        norm, _ = _hilo_pack(xt)
        x8s.append(norm.reshape(128, -1))
    in_maps = [dict(halves[c % 2], X8=x8s[c // 2]) for c in range(NCORES)]
    res = run_bass_kernel_spmd(nc, in_maps, list(range(NCORES)))
    out = np.empty((B, S, E), np.float32)
    for b in range(B):
        out[b] = (res.results[2 * b]["Y"].astype(np.float32)
                  + res.results[2 * b + 1]["Y"].astype(np.float32)
                  + bo[None, :])
    return out
